# revision 13
# baseline (speedup 1.0000x reference)
"""Trainium2 Bass kernel for nn_Attention_12970801234663.

Module: GroupNorm(32) -> 1x1 conv qkv -> 8-head attention over hw=1024 with the
original torch module's raw (b, heads, hw, head_dim) -> (b, c, h, w) reshape ->
1x1 out conv -> residual.

Sharding: pure data-parallel over batch (b=8) across 8 NeuronCores; weights are
broadcast. Each core computes one image end-to-end; no collectives.

Device-side plan (per core, c=256, hw=1024, heads=8, d=32), engineered against
the TimelineSim cost model (matmul cost = out-free-rows x cycles/row; fp8
DoubleRow = 0.5 cyc/row; ACT/DVE/Pool charge by free size):
  - GroupNorm stats via free-dim reduces + tiny PE matmuls against group
    indicator matrices; xn emitted in bf16 (tensor_scalar with per-channel
    A,B), which doubles as the qkv matmul operand conversion.
  - qkv projection in bf16. Pass A emits q,k with channels partition-packed
    as (32*(h%4)+d) so the fp8 DoubleRow sim can slice 32-aligned head
    blocks; eviction on ScalarE folds the qkv bias and converts to fp8e4
    into a zero-padded-double-row layout [128, hg, 2, 1024] (t=1 plane is
    zeros so DoubleRow's second k-tile contributes nothing).
  - sim[j,i] per (head, j-tile) via one fp8e4 DoubleRow matmul pair
    (N=512 each, 0.5 cyc/row) on 32-aligned partition blocks.
  - softmax exp with a constant -1.5 shift (cancels in the softmax ratio;
    keeps everything well inside bf16/fp8 range), statically split across
    three engines: ScalarE native Exp, and VectorE/GpSimd via a Schraudolph
    bf16 bit-trick (x*128/ln2 + 16251 -> int16 -> reinterpret bf16,
    ~2% rms).  All e tiles land in bf16.
  - attn@v in the transposed orientation out^T[i, (m|den)]: lhsT = e-chunk
    (stationary), rhs = [v^T | ones] (N=33) so output partitions are full
    (128 i's) and the softmax denominator rides along as one extra column.
  - softmax divide per head on VectorE/GpSimd, output bf16.
  - the module's scrambling reshape is a pure cross-partition collapse:
    one scatter DMA per head into a DRAM bounce in the scrambled channel
    order (64-byte runs), contiguous read-back per 128-channel tile.
  - out projection in bf16 with the (out_b + Wo@v-bias-pattern) term added
    via a rank-32 matmul against a (p%32) indicator, residual folded into
    the PSUM eviction op.
"""
import os
import sys

for p in ("/opt/trn_rl_repo",):
    if p not in sys.path and os.path.isdir(p):
        sys.path.insert(0, p)

import copy as _copy
import numpy as np
import ml_dtypes

import concourse.bass as bass
import concourse.tile as tile
from concourse import mybir
from concourse.bass_utils import run_bass_kernel_spmd
from concourse.bass_interp import get_hw_module

F32 = mybir.dt.float32
BF16 = mybir.dt.bfloat16
FP8E4 = mybir.dt.float8e4
I16 = mybir.dt.int16
ALU = mybir.AluOpType
AFT = mybir.ActivationFunctionType
PM = mybir.MatmulPerfMode

N_CORES = 8
B, C, H, W = 8, 256, 32, 32
HW = H * W                # 1024
N_HEADS = 8
HEAD_DIM = 32
GROUPS = 32
EPS = 1e-5
SCALE = HEAD_DIM ** -0.5
GROUP_SZ = (C // GROUPS) * HW  # 8192 elements per group

# softmax shift (cancels exactly in the softmax ratio)
ESHIFT = -1.5
# Schraudolph bf16 exp: bits = floor(x * 128/ln2 + 16251)
SCHR_SC = 128.0 / float(np.log(2.0))
SCHR_C = 16251.0 + ESHIFT * SCHR_SC

# fp32 consts columns
COL_NWB = 0      # 4: norm_w t0, norm_w t1, norm_b t0, norm_b t1
COL_GIND = 4     # 16: [128,16] group indicator
COL_GINDT = 20   # 128: rows 0:16 hold the [16,128] broadcast indicator
COL_QKB = 148    # 4: qk bias per pass-A psum tile (q0,q1,k0,k1), q scaled
COL_ESH = 152    # 1: ESHIFT broadcast column
CW = 153
# bf16 consts columns
CB_P32 = 0       # 1024: [32,1024] P32[r,p] = (p%32==r)
CB_YBT = 1024    # 256: [32, 2, 128] ybT[r, ot, o] = ybias[128*ot+o, r]
CWB = 1280

# exp engine assignment per (head, jtile): 'A'=ScalarE, 'D'=VectorE.
# (GPSIMD cannot touch PSUM, so it only gets SBUF-side work: xn + GN sums.)
EXP_ASSIGN = [['A', 'D', 'A', 'D', 'A', 'D', 'A', 'D'],
              ['D', 'A', 'D', 'A', 'D', 'A', 'D', 'A']] * 4
# v-eviction engine per hw-chunk
VEV_ASSIGN = ['A'] * 8
# divide engine per head
DIV_ASSIGN = ['D', 'D', 'D', 'D', 'D', 'D', 'D', 'D']


def _split_excess_waits(m):
    """Walrus in this toolchain accepts only one sem-wait per instruction;
    move excess waits onto preceding wait-only drains on the same engine."""
    n_split = 0
    for function in m.functions:
        new_blocks = []
        for block in function.blocks:
            new_insts = []
            for ins in block.instructions:
                si = ins.sync_info
                if si is None:
                    new_insts.append(ins)
                    continue
                waits = list(si.on_wait)
                if len(waits) > 1:
                    k = 0
                    while len(waits) > 1:
                        chunk, waits = waits[:1], waits[1:]
                        d = mybir.InstDrain(
                            name=f"{ins.name}-wsplit{k}",
                            ins=[], outs=[], bass_is_fusable=False,
                        )
                        d.engine = ins.engine
                        d.sync_info = mybir.SyncInfo(on_wait=chunk, on_update=[])
                        new_insts.append(d)
                        k += 1
                        n_split += 1
                    ins.sync_info = mybir.SyncInfo(
                        on_wait=waits, on_update=list(si.on_update))
                new_insts.append(ins)
            new_blocks.append(_copy.replace(block, instructions=new_insts))
        function.blocks.clear()
        function.blocks.extend(new_blocks)
    return n_split


def build_program(fix_for_hw=True):
    nc = bass.Bass("TRN2", target_bir_lowering=False, debug=False,
                   enable_asserts=False, num_devices=N_CORES)

    x_in = nc.dram_tensor("x_in", [128, 2, HW], F32, kind="ExternalInput")
    wall_in = nc.dram_tensor("wall", [128, 2, 1024], BF16, kind="ExternalInput")
    consts_in = nc.dram_tensor("consts", [128, CW], F32, kind="ExternalInput")
    constsb_in = nc.dram_tensor("constsb", [128, CWB], BF16,
                                kind="ExternalInput")
    zeros8_in = nc.dram_tensor("zeros8", [128, 2, HW], FP8E4,
                               kind="ExternalInput")
    y_out = nc.dram_tensor("y_out", [C, HW], F32, kind="ExternalOutput")

    ctx_lp = nc.allow_low_precision("bf16/fp8 attention by design")
    ctx_lp.__enter__()
    with tile.TileContext(nc) as tc:
        with (
            tc.tile_pool(name="persist", bufs=1) as persist,
            tc.tile_pool(name="ering", bufs=4) as ering,
            tc.tile_pool(name="scratch", bufs=2) as scratch,
            tc.tile_pool(name="psump", bufs=1, space="PSUM") as psump,
            tc.tile_pool(name="dramp", bufs=1, space="DRAM") as dramp,
        ):
            x_sb = persist.tile([128, 2, HW], F32)
            for t in range(2):
                nc.sync.dma_start(x_sb[:, t, :], x_in[:, t, :])
            consts = persist.tile([128, CW], F32)
            nc.sync.dma_start(consts[:], consts_in[:])
            wall = persist.tile([128, 2, 1024], BF16)
            nc.sync.dma_start(wall[:], wall_in[:])
            constsb = persist.tile([128, CWB], BF16)
            nc.sync.dma_start(constsb[:], constsb_in[:])

            # q/k fp8 zero-padded double-row layouts [128, hg, t, i]
            q8 = persist.tile([128, 2, 2, HW], FP8E4)
            k8 = persist.tile([128, 2, 2, HW], FP8E4)
            nc.sync.dma_start(q8[:, :, 1, :], zeros8_in[:])
            nc.sync.dma_start(k8[:, :, 1, :], zeros8_in[:])

            # [v^T | ones] per (head, jc) in bf16
            vaug = persist.tile([128, N_HEADS, 8, 33], BF16)
            nc.gpsimd.memset(vaug[:, :, :, 32:33], 1.0)

            # ---------------- GroupNorm ----------------
            ab_t = []
            for t in range(2):
                s_tile = scratch.tile([128, 2], F32, tag="gn_s")
                junk = scratch.tile([128, HW], F32, tag="junk")
                nc.scalar.activation(junk[:], x_sb[:, t, :], AFT.Copy,
                                     accum_out=s_tile[:, 0:1])
                junk2 = scratch.tile([128, HW], F32, tag="junk")
                nc.scalar.activation(junk2[:], x_sb[:, t, :], AFT.Square,
                                     accum_out=s_tile[:, 1:2])
                gsum = psump.tile([16, 2], F32, tag="sm", bufs=2)
                nc.tensor.matmul(gsum[:], consts[:, COL_GIND:COL_GIND + 16],
                                 s_tile[:])
                st = scratch.tile([16, 2], F32, tag="gn_st")
                nc.vector.tensor_scalar(st[:], gsum[:], 1.0 / GROUP_SZ, None,
                                        ALU.mult)
                mu_rs = scratch.tile([16, 2], F32, tag="gn_mr")
                nc.vector.tensor_copy(mu_rs[:, 0:1], st[:, 0:1])
                var_t = scratch.tile([16, 1], F32, tag="gn_var")
                nc.vector.tensor_tensor(var_t[:], st[:, 0:1], st[:, 0:1],
                                        ALU.mult)
                nc.vector.tensor_tensor(var_t[:], st[:, 1:2], var_t[:],
                                        ALU.subtract)
                nc.vector.tensor_scalar_add(var_t[:], var_t[:], EPS)
                ln_t = scratch.tile([16, 1], F32, tag="gn_ln")
                nc.scalar.activation(ln_t[:], var_t[:], AFT.Ln)
                nc.scalar.activation(mu_rs[:, 1:2], ln_t[:], AFT.Exp,
                                     scale=-0.5)
                bc = psump.tile([128, 2], F32, tag="sm", bufs=2)
                nc.tensor.matmul(bc[:], consts[0:16, COL_GINDT:COL_GINDT + 128],
                                 mu_rs[:])
                ab = scratch.tile([128, 2], F32, tag="gn_ab", bufs=2)
                # A = rsqrt * w ; B = b - mu * A
                nc.vector.tensor_tensor(ab[:, 0:1], bc[:, 1:2],
                                        consts[:, COL_NWB + t:COL_NWB + t + 1],
                                        ALU.mult)
                tmp_b = scratch.tile([128, 1], F32, tag="gn_tmp")
                nc.vector.tensor_tensor(tmp_b[:], bc[:, 0:1], ab[:, 0:1],
                                        ALU.mult)
                nc.vector.tensor_tensor(
                    ab[:, 1:2],
                    consts[:, COL_NWB + 2 + t:COL_NWB + 3 + t], tmp_b[:],
                    ALU.subtract)
                ab_t.append(ab)

            xn_bf = persist.tile([128, 2, HW], BF16)
            for t in range(2):
                # xn = x*A + B, emitted bf16 (GpSimd: SBUF->SBUF is legal there)
                nc.gpsimd.tensor_scalar(xn_bf[:, t, :], x_sb[:, t, :],
                                        ab_t[t][:, 0:1], ab_t[t][:, 1:2],
                                        ALU.mult, ALU.add)

            # ---------------- qkv pass A: q,k [channel, hw] ----------------
            # psum tile m: 0,1 = q hg0/hg1 ; 2,3 = k hg0/hg1 (natural order)
            for m in (0, 2, 1, 3):
                ps = psump.tile([128, 2, 512], F32, tag="big", bufs=3)
                for n in range(2):
                    for kc in range(2):
                        nc.tensor.matmul(
                            ps[:, n, :],
                            wall[:, kc, 128 * m:128 * (m + 1)],
                            xn_bf[:, kc, 512 * n:512 * (n + 1)],
                            start=(kc == 0), stop=(kc == 1))
                dst = q8 if m < 2 else k8
                nc.scalar.activation(
                    dst[:, m % 2, 0, :], ps[:].rearrange("p n f -> p (n f)"),
                    AFT.Identity, bias=consts[:, COL_QKB + m:COL_QKB + m + 1])

            # ---------------- qkv pass B: v as [hw, channel] ---------------
            for cch in range(8):
                psb = psump.tile([128, 256], F32, tag="sm", bufs=2)
                for kc in range(2):
                    nc.tensor.matmul(
                        psb[:],
                        xn_bf[:, kc, 128 * cch:128 * (cch + 1)],
                        wall[:, kc, 512:768], start=(kc == 0), stop=(kc == 1))
                vv = vaug[:, :, cch, 0:32]
                pv = psb[:].rearrange("p (h d) -> p h d", d=32)
                if VEV_ASSIGN[cch] == 'A':
                    nc.scalar.copy(vv, pv)
                else:
                    nc.vector.tensor_copy(vv, pv)

            # ---------------- attention ----------------
            a_drams = [dramp.tile([32, HW], BF16, tag=f"adram{h}",
                                  name=f"a_dram{h}")
                       for h in range(N_HEADS)]
            a_sb = persist.tile([128, 2, HW], BF16)
            e_tiles = {}

            def sim_exp_pair(hpair):
                for h in hpair:
                    e_tiles[h] = ering.tile([128, 8, HW], BF16, tag="e16",
                                            name=f"e16_{h}")
                for jt in range(8):
                    for h in hpair:
                        sim_exp_one(h, jt)

            def sim_exp_one(h, jt):
                b_, hg = h % 4, h // 4
                e16 = e_tiles[h]
                if True:
                    sim = psump.tile([128, 2, 512], F32, tag="big", bufs=3,
                                     name=f"sim_{h}_{jt}")
                    for n in range(2):
                        nc.tensor.matmul(
                            sim[:, n, :],
                            k8[32 * b_:32 * b_ + 32, hg, :,
                               128 * jt:128 * (jt + 1)],
                            q8[32 * b_:32 * b_ + 32, hg, :,
                               512 * n:512 * (n + 1)],
                            start=True, stop=True, perf_mode=PM.DoubleRow,
                            tile_position=(32 * b_, 0))
                    eng = EXP_ASSIGN[h][jt]
                    simf = sim[:].rearrange("p n f -> p (n f)")
                    if eng == 'A':
                        nc.scalar.activation(
                            e16[:, jt, :], simf, AFT.Exp,
                            bias=consts[:, COL_ESH:COL_ESH + 1])
                    else:
                        nc.vector.tensor_scalar(
                            e16[:, jt, :].bitcast(I16), simf,
                            SCHR_SC, SCHR_C, ALU.mult, ALU.add)

            def av_head(h):
                e16 = e_tiles.pop(h)
                avp = psump.tile([128, 8, 33], F32, tag="sm", bufs=2,
                                 name=f"avp_{h}")
                for it in range(8):
                    for jc in range(8):
                        nc.tensor.matmul(
                            avp[:, it, :],
                            e16[:, jc, 128 * it:128 * (it + 1)],
                            vaug[:, h, jc, :],
                            start=(jc == 0), stop=(jc == 7))
                recip = scratch.tile([128, 8], F32, tag="recip")
                nc.vector.reciprocal(recip[:], avp[:, :, 32])
                dv = scratch.tile([128, 8, 32], BF16, tag="avdiv", bufs=4,
                                  name=f"avdiv_{h}")
                rb = recip[:].unsqueeze(2).broadcast_to((128, 8, 32))
                nc.vector.tensor_tensor(dv[:], avp[:, :, 0:32], rb, ALU.mult)
                a_scat = a_drams[h][:].rearrange(
                    "(it g) (il d) -> g il it d", it=8, g=4, il=32)
                nc.sync.dma_start(a_scat, dv[:])
                nc.gpsimd.dma_start(
                    a_sb[32 * (h % 4):32 * (h % 4) + 32, h // 4, :],
                    a_drams[h][:])

            for p_ in range(5):
                if p_ < 4:
                    sim_exp_pair((2 * p_, 2 * p_ + 1))
                if p_ >= 1:
                    av_head(2 * p_ - 2)
                    av_head(2 * p_ - 1)

            # ---------------- out projection + residual ----------------
            for ot in range(2):
                y_sb = scratch.tile([128, HW], F32, tag="y_sb", bufs=2)
                for n in range(2):
                    ps = psump.tile([128, 512], F32, tag="big", bufs=3,
                                    name=f"op_{ot}_{n}")
                    for ct in range(2):
                        nc.tensor.matmul(
                            ps[:],
                            wall[:, ct, 768 + 128 * ot:768 + 128 * (ot + 1)],
                            a_sb[:, ct, 512 * n:512 * (n + 1)],
                            start=(ct == 0), stop=False)
                    nc.tensor.matmul(
                        ps[:],
                        constsb[0:32, CB_YBT + 128 * ot:CB_YBT + 128 * (ot + 1)],
                        constsb[0:32, CB_P32 + 512 * n:CB_P32 + 512 * (n + 1)],
                        start=False, stop=True)
                    nc.vector.tensor_tensor(
                        y_sb[:, 512 * n:512 * (n + 1)], ps[:],
                        x_sb[:, ot, 512 * n:512 * (n + 1)], ALU.add)
                    dma_eng = nc.sync if ot == 0 else nc.scalar
                    dma_eng.dma_start(
                        y_out[128 * ot:128 * (ot + 1),
                              512 * n:512 * (n + 1)],
                        y_sb[:, 512 * n:512 * (n + 1)])

    ctx_lp.__exit__(None, None, None)
    nc.finalize()
    if fix_for_hw:
        nc.m = get_hw_module(nc.m)
        _split_excess_waits(nc.m)
    return nc


def host_prep(x, norm_w, norm_b, qkv_w, qkv_b, out_w, out_b):
    """Build per-core input maps from full inputs."""
    x = np.asarray(x, np.float32)
    qkv_w = np.asarray(qkv_w, np.float32)
    qkv_b = np.asarray(qkv_b, np.float32)
    out_w = np.asarray(out_w, np.float32)
    out_b = np.asarray(out_b, np.float32)
    norm_w = np.asarray(norm_w, np.float32)
    norm_b = np.asarray(norm_b, np.float32)

    wT = np.ascontiguousarray(qkv_w.T)          # [256, 768] in-ch major
    wqk = wT[:, 0:512].copy()
    wqk[:, 0:256] *= SCALE
    bqk = qkv_b[0:512].copy()
    bqk[0:256] *= SCALE
    wv = wT[:, 512:768]
    bv = qkv_b[512:768]
    woT = out_w.T                               # [256 in, 256 out]

    wall = np.zeros((128, 2, 1024), np.float32)
    for kc in range(2):
        wall[:, kc, 0:512] = wqk[128 * kc:128 * (kc + 1), :]
        wall[:, kc, 512:768] = wv[128 * kc:128 * (kc + 1), :]
        wall[:, kc, 768:1024] = woT[128 * kc:128 * (kc + 1), :]

    consts = np.zeros((128, CW), np.float32)
    consts[:, COL_NWB + 0] = norm_w[0:128]
    consts[:, COL_NWB + 1] = norm_w[128:256]
    consts[:, COL_NWB + 2] = norm_b[0:128]
    consts[:, COL_NWB + 3] = norm_b[128:256]
    p = np.arange(128)
    consts[p, COL_GIND + p // 8] = 1.0
    consts[p // 8, COL_GINDT + p] = 1.0  # rows 0:16
    for m in range(4):
        consts[:, COL_QKB + m] = bqk[128 * m:128 * (m + 1)]
    consts[:, COL_ESH] = ESHIFT

    # ybias[o, r] = sum_c Wo[o, c] * bv[(c//32)*32 + r] + out_b[o]
    bvpat = np.zeros((256, 32), np.float32)
    for c in range(256):
        bvpat[c, :] = bv[(c // 32) * 32 + np.arange(32)]
    ybias = out_w @ bvpat + out_b[:, None]      # [256, 32]

    constsb = np.zeros((128, CWB), np.float32)
    pp = np.arange(1024)
    constsb[pp % 32, CB_P32 + pp] = 1.0          # rows 0:32
    for ot in range(2):
        constsb[0:32, CB_YBT + 128 * ot:CB_YBT + 128 * (ot + 1)] = \
            ybias[128 * ot:128 * (ot + 1), :].T

    shared = {
        "wall": wall.astype(ml_dtypes.bfloat16),
        "consts": consts,
        "constsb": constsb.astype(ml_dtypes.bfloat16),
        "zeros8": np.zeros((128, 2, HW), ml_dtypes.float8_e4m3),
    }
    in_maps = []
    for b in range(N_CORES):
        m = dict(shared)
        m["x_in"] = np.ascontiguousarray(
            x[b].reshape(2, 128, HW).transpose(1, 0, 2))
        in_maps.append(m)
    return in_maps


_PROGRAM = None


def _get_program():
    global _PROGRAM
    if _PROGRAM is None:
        _PROGRAM = build_program()
    return _PROGRAM


def kernel(x, norm_w, norm_b, qkv_w, qkv_b, out_w, out_b, _trace=False):
    nc = _get_program()
    in_maps = host_prep(x, norm_w, norm_b, qkv_w, qkv_b, out_w, out_b)
    res = run_bass_kernel_spmd(nc, in_maps, list(range(N_CORES)), trace=_trace)
    out = np.stack([res.results[b]["y_out"].reshape(C, H, W)
                    for b in range(N_CORES)])
    if _trace:
        kernel.last_result = res
    return out.astype(np.float32)


# revision 16
# speedup vs baseline: 1.0037x; 1.0037x over previous
"""Trainium2 Bass kernel for nn_Attention_12970801234663.

Module: GroupNorm(32) -> 1x1 conv qkv -> 8-head attention over hw=1024 with the
original torch module's raw (b, heads, hw, head_dim) -> (b, c, h, w) reshape ->
1x1 out conv -> residual.

Sharding: pure data-parallel over batch (b=8) across 8 NeuronCores; weights are
broadcast. Each core computes one image end-to-end; no collectives.

Device-side plan (per core, c=256, hw=1024, heads=8, d=32), engineered against
the TimelineSim cost model (matmul cost = out-free-rows x cycles/row; fp8
DoubleRow = 0.5 cyc/row; ACT/DVE/Pool charge by free size):
  - GroupNorm stats via free-dim reduces + tiny PE matmuls against group
    indicator matrices; xn emitted in bf16 (tensor_scalar with per-channel
    A,B), which doubles as the qkv matmul operand conversion.
  - qkv projection in bf16. Pass A emits q,k with channels partition-packed
    as (32*(h%4)+d) so the fp8 DoubleRow sim can slice 32-aligned head
    blocks; eviction on ScalarE folds the qkv bias and converts to fp8e4
    into a zero-padded-double-row layout [128, hg, 2, 1024] (t=1 plane is
    zeros so DoubleRow's second k-tile contributes nothing).
  - sim[j,i] per (head, j-tile) via one fp8e4 DoubleRow matmul pair
    (N=512 each, 0.5 cyc/row) on 32-aligned partition blocks.
  - softmax exp with a constant -1.5 shift (cancels in the softmax ratio;
    keeps everything well inside bf16/fp8 range), statically split across
    three engines: ScalarE native Exp, and VectorE/GpSimd via a Schraudolph
    bf16 bit-trick (x*128/ln2 + 16251 -> int16 -> reinterpret bf16,
    ~2% rms).  All e tiles land in bf16.
  - attn@v in the transposed orientation out^T[i, (m|den)]: lhsT = e-chunk
    (stationary), rhs = [v^T | ones] (N=33) so output partitions are full
    (128 i's) and the softmax denominator rides along as one extra column.
  - softmax divide per head on VectorE/GpSimd, output bf16.
  - the module's scrambling reshape is a pure cross-partition collapse:
    one scatter DMA per head into a DRAM bounce in the scrambled channel
    order (64-byte runs), contiguous read-back per 128-channel tile.
  - out projection in bf16 with the (out_b + Wo@v-bias-pattern) term added
    via a rank-32 matmul against a (p%32) indicator, residual folded into
    the PSUM eviction op.
"""
import os
import sys

for p in ("/opt/trn_rl_repo",):
    if p not in sys.path and os.path.isdir(p):
        sys.path.insert(0, p)

import copy as _copy
import numpy as np
import ml_dtypes

import concourse.bass as bass
import concourse.tile as tile
from concourse import mybir
from concourse.bass_utils import run_bass_kernel_spmd
from concourse.bass_interp import get_hw_module

F32 = mybir.dt.float32
BF16 = mybir.dt.bfloat16
FP8E4 = mybir.dt.float8e4
I16 = mybir.dt.int16
ALU = mybir.AluOpType
AFT = mybir.ActivationFunctionType
PM = mybir.MatmulPerfMode

N_CORES = 8
B, C, H, W = 8, 256, 32, 32
HW = H * W                # 1024
N_HEADS = 8
HEAD_DIM = 32
GROUPS = 32
EPS = 1e-5
SCALE = HEAD_DIM ** -0.5
GROUP_SZ = (C // GROUPS) * HW  # 8192 elements per group

# softmax shift (cancels exactly in the softmax ratio)
ESHIFT = -1.5
# Schraudolph bf16 exp: bits = floor(x * 128/ln2 + 16251)
SCHR_SC = 128.0 / float(np.log(2.0))
SCHR_C = 16251.0 + ESHIFT * SCHR_SC

# fp32 consts columns
COL_NWB = 0      # 4: norm_w t0, norm_w t1, norm_b t0, norm_b t1
COL_GIND = 4     # 16: [128,16] group indicator
COL_GINDT = 20   # 128: rows 0:16 hold the [16,128] broadcast indicator
COL_QKB = 148    # 4: qk bias per pass-A psum tile (q0,q1,k0,k1), q scaled
COL_ESH = 152    # 1: ESHIFT broadcast column
COL_EPS = 153    # 1: GroupNorm eps broadcast column
CW = 154
# bf16 consts columns
CB_P32 = 0       # 1024: [32,1024] P32[r,p] = (p%32==r)
CB_YBT = 1024    # 256: [32, 2, 128] ybT[r, ot, o] = ybias[128*ot+o, r]
CWB = 1280

# exp engine assignment per (head, jtile): 'A'=ScalarE, 'D'=VectorE.
# (GPSIMD cannot touch PSUM, so it only gets SBUF-side work: xn + GN sums.)
EXP_ASSIGN = [['A', 'D', 'A', 'D', 'A', 'D', 'A', 'D'],
              ['D', 'A', 'D', 'A', 'D', 'A', 'D', 'A']] * 4
# v-eviction engine per hw-chunk
VEV_ASSIGN = ['A'] * 8
# divide engine per head
DIV_ASSIGN = ['D', 'D', 'D', 'D', 'D', 'D', 'D', 'D']


def _split_excess_waits(m):
    """Walrus in this toolchain accepts only one sem-wait per instruction;
    move excess waits onto preceding wait-only drains on the same engine."""
    n_split = 0
    for function in m.functions:
        new_blocks = []
        for block in function.blocks:
            new_insts = []
            for ins in block.instructions:
                si = ins.sync_info
                if si is None:
                    new_insts.append(ins)
                    continue
                waits = list(si.on_wait)
                if len(waits) > 1:
                    k = 0
                    while len(waits) > 1:
                        chunk, waits = waits[:1], waits[1:]
                        d = mybir.InstDrain(
                            name=f"{ins.name}-wsplit{k}",
                            ins=[], outs=[], bass_is_fusable=False,
                        )
                        d.engine = ins.engine
                        d.sync_info = mybir.SyncInfo(on_wait=chunk, on_update=[])
                        new_insts.append(d)
                        k += 1
                        n_split += 1
                    ins.sync_info = mybir.SyncInfo(
                        on_wait=waits, on_update=list(si.on_update))
                new_insts.append(ins)
            new_blocks.append(_copy.replace(block, instructions=new_insts))
        function.blocks.clear()
        function.blocks.extend(new_blocks)
    return n_split


def build_program(fix_for_hw=True):
    nc = bass.Bass("TRN2", target_bir_lowering=False, debug=False,
                   enable_asserts=False, num_devices=N_CORES)

    x_in = nc.dram_tensor("x_in", [128, 2, HW], F32, kind="ExternalInput")
    wall_in = nc.dram_tensor("wall", [128, 2, 1024], BF16, kind="ExternalInput")
    consts_in = nc.dram_tensor("consts", [128, CW], F32, kind="ExternalInput")
    constsb_in = nc.dram_tensor("constsb", [128, CWB], BF16,
                                kind="ExternalInput")
    zeros8_in = nc.dram_tensor("zeros8", [128, 2, HW], FP8E4,
                               kind="ExternalInput")
    y_out = nc.dram_tensor("y_out", [C, HW], F32, kind="ExternalOutput")

    ctx_lp = nc.allow_low_precision("bf16/fp8 attention by design")
    ctx_lp.__enter__()
    with tile.TileContext(nc) as tc:
        with (
            tc.tile_pool(name="persist", bufs=1) as persist,
            tc.tile_pool(name="ering", bufs=4) as ering,
            tc.tile_pool(name="scratch", bufs=2) as scratch,
            tc.tile_pool(name="psump", bufs=1, space="PSUM") as psump,
            tc.tile_pool(name="dramp", bufs=1, space="DRAM") as dramp,
        ):
            x_sb = persist.tile([128, 2, HW], F32)
            for t in range(2):
                nc.sync.dma_start(x_sb[:, t, :], x_in[:, t, :])
            consts = persist.tile([128, CW], F32)
            nc.sync.dma_start(consts[:], consts_in[:])
            wall = persist.tile([128, 2, 1024], BF16)
            nc.sync.dma_start(wall[:], wall_in[:])
            constsb = persist.tile([128, CWB], BF16)
            nc.sync.dma_start(constsb[:], constsb_in[:])

            # q/k fp8 zero-padded double-row layouts [128, hg, t, i]
            q8 = persist.tile([128, 2, 2, HW], FP8E4)
            k8 = persist.tile([128, 2, 2, HW], FP8E4)
            nc.sync.dma_start(q8[:, :, 1, :], zeros8_in[:])
            nc.sync.dma_start(k8[:, :, 1, :], zeros8_in[:])

            # [v^T | ones] per (head, jc) in bf16
            vaug = persist.tile([128, N_HEADS, 8, 33], BF16)
            nc.gpsimd.memset(vaug[:, :, :, 32:33], 1.0)

            # ---------------- GroupNorm ----------------
            ab_t = []
            for t in range(2):
                s_tile = scratch.tile([128, 2], F32, tag="gn_s")
                junk = scratch.tile([128, HW], F32, tag="junk")
                nc.scalar.activation(junk[:], x_sb[:, t, :], AFT.Copy,
                                     accum_out=s_tile[:, 0:1])
                junk2 = scratch.tile([128, HW], F32, tag="junk")
                nc.scalar.activation(junk2[:], x_sb[:, t, :], AFT.Square,
                                     accum_out=s_tile[:, 1:2])
                gsum = psump.tile([16, 2], F32, tag="sm", bufs=2)
                nc.tensor.matmul(gsum[:], consts[:, COL_GIND:COL_GIND + 16],
                                 s_tile[:])
                mu_rs = scratch.tile([16, 2], F32, tag="gn_mr")
                nc.vector.tensor_copy(mu_rs[:, 0:1], gsum[:, 0:1])
                var_t = scratch.tile([16, 1], F32, tag="gn_var")
                nc.vector.tensor_tensor(var_t[:], mu_rs[:, 0:1],
                                        mu_rs[:, 0:1], ALU.mult)
                nc.vector.tensor_tensor(var_t[:], gsum[:, 1:2], var_t[:],
                                        ALU.subtract)
                ln_t = scratch.tile([16, 1], F32, tag="gn_ln")
                nc.scalar.activation(ln_t[:], var_t[:], AFT.Ln,
                                     bias=consts[0:16, COL_EPS:COL_EPS + 1])
                nc.scalar.activation(mu_rs[:, 1:2], ln_t[:], AFT.Exp,
                                     scale=-0.5)
                bc = psump.tile([128, 2], F32, tag="sm", bufs=2)
                nc.tensor.matmul(bc[:], consts[0:16, COL_GINDT:COL_GINDT + 128],
                                 mu_rs[:])
                ab = scratch.tile([128, 2], F32, tag="gn_ab", bufs=2)
                # A = rsqrt * w ; B = b - mu * A
                nc.vector.tensor_tensor(ab[:, 0:1], bc[:, 1:2],
                                        consts[:, COL_NWB + t:COL_NWB + t + 1],
                                        ALU.mult)
                tmp_b = scratch.tile([128, 1], F32, tag="gn_tmp")
                nc.vector.tensor_tensor(tmp_b[:], bc[:, 0:1], ab[:, 0:1],
                                        ALU.mult)
                nc.vector.tensor_tensor(
                    ab[:, 1:2],
                    consts[:, COL_NWB + 2 + t:COL_NWB + 3 + t], tmp_b[:],
                    ALU.subtract)
                ab_t.append(ab)

            xn_bf = persist.tile([128, 2, HW], BF16)
            for t in range(2):
                # xn = x*A + B, emitted bf16 (GpSimd: SBUF->SBUF is legal there)
                nc.gpsimd.tensor_scalar(xn_bf[:, t, :], x_sb[:, t, :],
                                        ab_t[t][:, 0:1], ab_t[t][:, 1:2],
                                        ALU.mult, ALU.add)

            # ---------------- qkv pass A: q,k [channel, hw] ----------------
            # psum tile m: 0,1 = q hg0/hg1 ; 2,3 = k hg0/hg1 (natural order)
            for m in (0, 2, 1, 3):
                ps = psump.tile([128, 2, 512], F32, tag="big", bufs=3)
                for n in range(2):
                    for kc in range(2):
                        nc.tensor.matmul(
                            ps[:, n, :],
                            wall[:, kc, 128 * m:128 * (m + 1)],
                            xn_bf[:, kc, 512 * n:512 * (n + 1)],
                            start=(kc == 0), stop=(kc == 1))
                dst = q8 if m < 2 else k8
                nc.scalar.activation(
                    dst[:, m % 2, 0, :], ps[:].rearrange("p n f -> p (n f)"),
                    AFT.Identity, bias=consts[:, COL_QKB + m:COL_QKB + m + 1])

            # ---------------- qkv pass B: v as [hw, channel] ---------------
            for cch in range(8):
                psb = psump.tile([128, 256], F32, tag="sm", bufs=2)
                for kc in range(2):
                    nc.tensor.matmul(
                        psb[:],
                        xn_bf[:, kc, 128 * cch:128 * (cch + 1)],
                        wall[:, kc, 512:768], start=(kc == 0), stop=(kc == 1))
                vv = vaug[:, :, cch, 0:32]
                pv = psb[:].rearrange("p (h d) -> p h d", d=32)
                if VEV_ASSIGN[cch] == 'A':
                    nc.scalar.copy(vv, pv)
                else:
                    nc.vector.tensor_copy(vv, pv)

            # ---------------- attention ----------------
            a_drams = [dramp.tile([32, HW], BF16, tag=f"adram{h}",
                                  name=f"a_dram{h}")
                       for h in range(N_HEADS)]
            a_sb = persist.tile([128, 2, HW], BF16)
            e_tiles = {}

            def sim_exp_pair(hpair):
                for h in hpair:
                    e_tiles[h] = ering.tile([128, 8, HW], BF16, tag="e16",
                                            name=f"e16_{h}")
                for jt in range(8):
                    for h in hpair:
                        sim_exp_one(h, jt)

            def sim_exp_one(h, jt):
                b_, hg = h % 4, h // 4
                e16 = e_tiles[h]
                if True:
                    sim = psump.tile([128, 2, 512], F32, tag="big", bufs=3,
                                     name=f"sim_{h}_{jt}")
                    for n in range(2):
                        nc.tensor.matmul(
                            sim[:, n, :],
                            k8[32 * b_:32 * b_ + 32, hg, :,
                               128 * jt:128 * (jt + 1)],
                            q8[32 * b_:32 * b_ + 32, hg, :,
                               512 * n:512 * (n + 1)],
                            start=True, stop=True, perf_mode=PM.DoubleRow,
                            tile_position=(32 * b_, 0))
                    eng = EXP_ASSIGN[h][jt]
                    simf = sim[:].rearrange("p n f -> p (n f)")
                    if eng == 'A':
                        nc.scalar.activation(
                            e16[:, jt, :], simf, AFT.Exp,
                            bias=consts[:, COL_ESH:COL_ESH + 1])
                    else:
                        nc.vector.tensor_scalar(
                            e16[:, jt, :].bitcast(I16), simf,
                            SCHR_SC, SCHR_C, ALU.mult, ALU.add)

            def av_head(h):
                e16 = e_tiles.pop(h)
                avp = psump.tile([128, 8, 33], F32, tag="sm", bufs=2,
                                 name=f"avp_{h}")
                for it in range(8):
                    for jc in range(8):
                        nc.tensor.matmul(
                            avp[:, it, :],
                            e16[:, jc, 128 * it:128 * (it + 1)],
                            vaug[:, h, jc, :],
                            start=(jc == 0), stop=(jc == 7))
                recip = scratch.tile([128, 8], F32, tag="recip")
                nc.vector.reciprocal(recip[:], avp[:, :, 32])
                dv = scratch.tile([128, 8, 32], BF16, tag="avdiv", bufs=4,
                                  name=f"avdiv_{h}")
                rb = recip[:].unsqueeze(2).broadcast_to((128, 8, 32))
                nc.vector.tensor_tensor(dv[:], avp[:, :, 0:32], rb, ALU.mult)
                a_scat = a_drams[h][:].rearrange(
                    "(it g) (il d) -> g il it d", it=8, g=4, il=32)
                nc.sync.dma_start(a_scat, dv[:])
                nc.gpsimd.dma_start(
                    a_sb[32 * (h % 4):32 * (h % 4) + 32, h // 4, :],
                    a_drams[h][:])

            for p_ in range(5):
                if p_ < 4:
                    sim_exp_pair((2 * p_, 2 * p_ + 1))
                if p_ >= 1:
                    av_head(2 * p_ - 2)
                    av_head(2 * p_ - 1)

            # ---------------- out projection + residual ----------------
            for ot in range(2):
                y_sb = scratch.tile([128, HW], F32, tag="y_sb", bufs=2)
                for n in range(2):
                    ps = psump.tile([128, 512], F32, tag="sm", bufs=2,
                                    name=f"op_{ot}_{n}")
                    for ct in range(2):
                        nc.tensor.matmul(
                            ps[:],
                            wall[:, ct, 768 + 128 * ot:768 + 128 * (ot + 1)],
                            a_sb[:, ct, 512 * n:512 * (n + 1)],
                            start=(ct == 0), stop=False)
                    nc.tensor.matmul(
                        ps[:],
                        constsb[0:32, CB_YBT + 128 * ot:CB_YBT + 128 * (ot + 1)],
                        constsb[0:32, CB_P32 + 512 * n:CB_P32 + 512 * (n + 1)],
                        start=False, stop=True)
                    nc.vector.tensor_tensor(
                        y_sb[:, 512 * n:512 * (n + 1)], ps[:],
                        x_sb[:, ot, 512 * n:512 * (n + 1)], ALU.add)
                    dma_eng = nc.sync if ot == 0 else nc.scalar
                    dma_eng.dma_start(
                        y_out[128 * ot:128 * (ot + 1),
                              512 * n:512 * (n + 1)],
                        y_sb[:, 512 * n:512 * (n + 1)])

    ctx_lp.__exit__(None, None, None)
    nc.finalize()
    if fix_for_hw:
        nc.m = get_hw_module(nc.m)
        _split_excess_waits(nc.m)
    return nc


def host_prep(x, norm_w, norm_b, qkv_w, qkv_b, out_w, out_b):
    """Build per-core input maps from full inputs."""
    x = np.asarray(x, np.float32)
    qkv_w = np.asarray(qkv_w, np.float32)
    qkv_b = np.asarray(qkv_b, np.float32)
    out_w = np.asarray(out_w, np.float32)
    out_b = np.asarray(out_b, np.float32)
    norm_w = np.asarray(norm_w, np.float32)
    norm_b = np.asarray(norm_b, np.float32)

    wT = np.ascontiguousarray(qkv_w.T)          # [256, 768] in-ch major
    wqk = wT[:, 0:512].copy()
    wqk[:, 0:256] *= SCALE
    bqk = qkv_b[0:512].copy()
    bqk[0:256] *= SCALE
    wv = wT[:, 512:768]
    bv = qkv_b[512:768]
    woT = out_w.T                               # [256 in, 256 out]

    wall = np.zeros((128, 2, 1024), np.float32)
    for kc in range(2):
        wall[:, kc, 0:512] = wqk[128 * kc:128 * (kc + 1), :]
        wall[:, kc, 512:768] = wv[128 * kc:128 * (kc + 1), :]
        wall[:, kc, 768:1024] = woT[128 * kc:128 * (kc + 1), :]

    consts = np.zeros((128, CW), np.float32)
    consts[:, COL_NWB + 0] = norm_w[0:128]
    consts[:, COL_NWB + 1] = norm_w[128:256]
    consts[:, COL_NWB + 2] = norm_b[0:128]
    consts[:, COL_NWB + 3] = norm_b[128:256]
    p = np.arange(128)
    consts[p, COL_GIND + p // 8] = 1.0 / GROUP_SZ
    consts[p // 8, COL_GINDT + p] = 1.0  # rows 0:16
    for m in range(4):
        consts[:, COL_QKB + m] = bqk[128 * m:128 * (m + 1)]
    consts[:, COL_ESH] = ESHIFT
    consts[:, COL_EPS] = EPS

    # ybias[o, r] = sum_c Wo[o, c] * bv[(c//32)*32 + r] + out_b[o]
    bvpat = np.zeros((256, 32), np.float32)
    for c in range(256):
        bvpat[c, :] = bv[(c // 32) * 32 + np.arange(32)]
    ybias = out_w @ bvpat + out_b[:, None]      # [256, 32]

    constsb = np.zeros((128, CWB), np.float32)
    pp = np.arange(1024)
    constsb[pp % 32, CB_P32 + pp] = 1.0          # rows 0:32
    for ot in range(2):
        constsb[0:32, CB_YBT + 128 * ot:CB_YBT + 128 * (ot + 1)] = \
            ybias[128 * ot:128 * (ot + 1), :].T

    shared = {
        "wall": wall.astype(ml_dtypes.bfloat16),
        "consts": consts,
        "constsb": constsb.astype(ml_dtypes.bfloat16),
        "zeros8": np.zeros((128, 2, HW), ml_dtypes.float8_e4m3),
    }
    in_maps = []
    for b in range(N_CORES):
        m = dict(shared)
        m["x_in"] = np.ascontiguousarray(
            x[b].reshape(2, 128, HW).transpose(1, 0, 2))
        in_maps.append(m)
    return in_maps


_PROGRAM = None


def _get_program():
    global _PROGRAM
    if _PROGRAM is None:
        _PROGRAM = build_program()
    return _PROGRAM


def kernel(x, norm_w, norm_b, qkv_w, qkv_b, out_w, out_b, _trace=False):
    nc = _get_program()
    in_maps = host_prep(x, norm_w, norm_b, qkv_w, qkv_b, out_w, out_b)
    res = run_bass_kernel_spmd(nc, in_maps, list(range(N_CORES)), trace=_trace)
    out = np.stack([res.results[b]["y_out"].reshape(C, H, W)
                    for b in range(N_CORES)])
    if _trace:
        kernel.last_result = res
    return out.astype(np.float32)


# revision 17
# speedup vs baseline: 1.0194x; 1.0156x over previous
"""Trainium2 Bass kernel for nn_Attention_12970801234663.

Module: GroupNorm(32) -> 1x1 conv qkv -> 8-head attention over hw=1024 with the
original torch module's raw (b, heads, hw, head_dim) -> (b, c, h, w) reshape ->
1x1 out conv -> residual.

Sharding: pure data-parallel over batch (b=8) across 8 NeuronCores; weights are
broadcast. Each core computes one image end-to-end; no collectives.

Device-side plan (per core, c=256, hw=1024, heads=8, d=32), engineered against
the TimelineSim cost model (matmul cost = out-free-rows x cycles/row; fp8
DoubleRow = 0.5 cyc/row; ACT/DVE/Pool charge by free size):
  - GroupNorm stats via free-dim reduces + tiny PE matmuls against group
    indicator matrices; xn emitted in bf16 (tensor_scalar with per-channel
    A,B), which doubles as the qkv matmul operand conversion.
  - qkv projection in bf16. Pass A emits q,k with channels partition-packed
    as (32*(h%4)+d) so the fp8 DoubleRow sim can slice 32-aligned head
    blocks; eviction on ScalarE folds the qkv bias and converts to fp8e4
    into a zero-padded-double-row layout [128, hg, 2, 1024] (t=1 plane is
    zeros so DoubleRow's second k-tile contributes nothing).
  - sim[j,i] per (head, j-tile) via one fp8e4 DoubleRow matmul pair
    (N=512 each, 0.5 cyc/row) on 32-aligned partition blocks.
  - softmax exp with a constant -1.5 shift (cancels in the softmax ratio;
    keeps everything well inside bf16/fp8 range), statically split across
    three engines: ScalarE native Exp, and VectorE/GpSimd via a Schraudolph
    bf16 bit-trick (x*128/ln2 + 16251 -> int16 -> reinterpret bf16,
    ~2% rms).  All e tiles land in bf16.
  - attn@v in the transposed orientation out^T[i, (m|den)]: lhsT = e-chunk
    (stationary), rhs = [v^T | ones] (N=33) so output partitions are full
    (128 i's) and the softmax denominator rides along as one extra column.
  - softmax divide per head on VectorE/GpSimd, output bf16.
  - the module's scrambling reshape is a pure cross-partition collapse:
    one scatter DMA per head into a DRAM bounce in the scrambled channel
    order (64-byte runs), contiguous read-back per 128-channel tile.
  - out projection in bf16 with the (out_b + Wo@v-bias-pattern) term added
    via a rank-32 matmul against a (p%32) indicator, residual folded into
    the PSUM eviction op.
"""
import os
import sys

for p in ("/opt/trn_rl_repo",):
    if p not in sys.path and os.path.isdir(p):
        sys.path.insert(0, p)

import copy as _copy
import numpy as np
import ml_dtypes

import concourse.bass as bass
import concourse.tile as tile
from concourse import mybir
from concourse.bass_utils import run_bass_kernel_spmd
from concourse.bass_interp import get_hw_module

F32 = mybir.dt.float32
BF16 = mybir.dt.bfloat16
FP8E4 = mybir.dt.float8e4
I16 = mybir.dt.int16
ALU = mybir.AluOpType
AFT = mybir.ActivationFunctionType
PM = mybir.MatmulPerfMode

N_CORES = 8
B, C, H, W = 8, 256, 32, 32
HW = H * W                # 1024
N_HEADS = 8
HEAD_DIM = 32
GROUPS = 32
EPS = 1e-5
SCALE = HEAD_DIM ** -0.5
GROUP_SZ = (C // GROUPS) * HW  # 8192 elements per group

# softmax shift (cancels exactly in the softmax ratio)
ESHIFT = -1.5
# Schraudolph bf16 exp: bits = floor(x * 128/ln2 + 16251)
SCHR_SC = 128.0 / float(np.log(2.0))
SCHR_C = 16251.0 + ESHIFT * SCHR_SC

# fp32 consts columns
COL_NWB = 0      # 4: norm_w t0, norm_w t1, norm_b t0, norm_b t1
COL_GIND = 4     # 16: [128,16] group indicator
COL_GINDT = 20   # 128: rows 0:16 hold the [16,128] broadcast indicator
COL_QKB = 148    # 4: qk bias per pass-A psum tile (q0,q1,k0,k1), q scaled
COL_ESH = 152    # 1: ESHIFT broadcast column
COL_EPS = 153    # 1: GroupNorm eps broadcast column
CW = 154
# bf16 consts columns
CB_P32 = 0       # 1024: [32,1024] P32[r,p] = (p%32==r)
CB_YBT = 1024    # 256: [32, 2, 128] ybT[r, ot, o] = ybias[128*ot+o, r]
CWB = 1280

# exp engine assignment per (head, jtile): 'A'=ScalarE, 'D'=VectorE.
# (GPSIMD cannot touch PSUM, so it only gets SBUF-side work: xn + GN sums.)
EXP_ASSIGN = [['A', 'D', 'A', 'D', 'A', 'D', 'A', 'D'],
              ['D', 'A', 'D', 'A', 'D', 'A', 'D', 'A']] * 4
# v-eviction engine per hw-chunk
VEV_ASSIGN = ['A'] * 8
# divide engine per head
DIV_ASSIGN = ['D', 'D', 'D', 'D', 'D', 'D', 'D', 'D']


def _split_excess_waits(m):
    """Walrus in this toolchain accepts only one sem-wait per instruction;
    move excess waits onto preceding wait-only drains on the same engine."""
    n_split = 0
    for function in m.functions:
        new_blocks = []
        for block in function.blocks:
            new_insts = []
            for ins in block.instructions:
                si = ins.sync_info
                if si is None:
                    new_insts.append(ins)
                    continue
                waits = list(si.on_wait)
                if len(waits) > 1:
                    k = 0
                    while len(waits) > 1:
                        chunk, waits = waits[:1], waits[1:]
                        d = mybir.InstDrain(
                            name=f"{ins.name}-wsplit{k}",
                            ins=[], outs=[], bass_is_fusable=False,
                        )
                        d.engine = ins.engine
                        d.sync_info = mybir.SyncInfo(on_wait=chunk, on_update=[])
                        new_insts.append(d)
                        k += 1
                        n_split += 1
                    ins.sync_info = mybir.SyncInfo(
                        on_wait=waits, on_update=list(si.on_update))
                new_insts.append(ins)
            new_blocks.append(_copy.replace(block, instructions=new_insts))
        function.blocks.clear()
        function.blocks.extend(new_blocks)
    return n_split


def build_program(fix_for_hw=True):
    nc = bass.Bass("TRN2", target_bir_lowering=False, debug=False,
                   enable_asserts=False, num_devices=N_CORES)

    x_in = nc.dram_tensor("x_in", [128, 2, HW], F32, kind="ExternalInput")
    wall_in = nc.dram_tensor("wall", [128, 2, 1024], BF16, kind="ExternalInput")
    consts_in = nc.dram_tensor("consts", [128, CW], F32, kind="ExternalInput")
    constsb_in = nc.dram_tensor("constsb", [128, CWB], BF16,
                                kind="ExternalInput")
    zeros8_in = nc.dram_tensor("zeros8", [128, 2, HW], FP8E4,
                               kind="ExternalInput")
    y_out = nc.dram_tensor("y_out", [C, HW], F32, kind="ExternalOutput")

    ctx_lp = nc.allow_low_precision("bf16/fp8 attention by design")
    ctx_lp.__enter__()
    with tile.TileContext(nc) as tc:
        with (
            tc.tile_pool(name="persist", bufs=1) as persist,
            tc.tile_pool(name="ering", bufs=4) as ering,
            tc.tile_pool(name="scratch", bufs=2) as scratch,
            tc.tile_pool(name="psump", bufs=1, space="PSUM") as psump,
            tc.tile_pool(name="dramp", bufs=1, space="DRAM") as dramp,
        ):
            x_sb = persist.tile([128, 2, HW], F32)
            for t in range(2):
                nc.sync.dma_start(x_sb[:, t, :], x_in[:, t, :])
            consts = persist.tile([128, CW], F32)
            nc.sync.dma_start(consts[:], consts_in[:])
            wall = persist.tile([128, 2, 1024], BF16)
            nc.sync.dma_start(wall[:], wall_in[:])
            constsb = persist.tile([128, CWB], BF16)
            nc.sync.dma_start(constsb[:], constsb_in[:])

            # q/k fp8 zero-padded double-row layouts [128, hg, t, i]
            q8 = persist.tile([128, 2, 2, HW], FP8E4)
            k8 = persist.tile([128, 2, 2, HW], FP8E4)
            nc.sync.dma_start(q8[:, :, 1, :], zeros8_in[:])
            nc.sync.dma_start(k8[:, :, 1, :], zeros8_in[:])

            # [v^T | ones] per (head, jc) in bf16
            vaug = persist.tile([128, N_HEADS, 8, 33], BF16)
            nc.gpsimd.memset(vaug[:, :, :, 32:33], 1.0)

            # ---------------- GroupNorm ----------------
            ab_t = []
            for t in range(2):
                s_tile = scratch.tile([128, 2], F32, tag="gn_s")
                junk = scratch.tile([128, HW], F32, tag="junk")
                nc.scalar.activation(junk[:], x_sb[:, t, :], AFT.Copy,
                                     accum_out=s_tile[:, 0:1])
                junk2 = scratch.tile([128, HW], F32, tag="junk")
                nc.scalar.activation(junk2[:], x_sb[:, t, :], AFT.Square,
                                     accum_out=s_tile[:, 1:2])
                gsum = psump.tile([16, 2], F32, tag="sm", bufs=2)
                nc.tensor.matmul(gsum[:], consts[:, COL_GIND:COL_GIND + 16],
                                 s_tile[:])
                mu_rs = scratch.tile([16, 2], F32, tag="gn_mr")
                nc.vector.tensor_copy(mu_rs[:, 0:1], gsum[:, 0:1])
                var_t = scratch.tile([16, 1], F32, tag="gn_var")
                nc.vector.tensor_tensor(var_t[:], mu_rs[:, 0:1],
                                        mu_rs[:, 0:1], ALU.mult)
                nc.vector.tensor_tensor(var_t[:], gsum[:, 1:2], var_t[:],
                                        ALU.subtract)
                ln_t = scratch.tile([16, 1], F32, tag="gn_ln")
                nc.scalar.activation(ln_t[:], var_t[:], AFT.Ln,
                                     bias=consts[0:16, COL_EPS:COL_EPS + 1])
                nc.scalar.activation(mu_rs[:, 1:2], ln_t[:], AFT.Exp,
                                     scale=-0.5)
                bc = psump.tile([128, 2], F32, tag="sm", bufs=2)
                nc.tensor.matmul(bc[:], consts[0:16, COL_GINDT:COL_GINDT + 128],
                                 mu_rs[:])
                ab = scratch.tile([128, 2], F32, tag="gn_ab", bufs=2)
                # A = rsqrt * w ; B = b - mu * A
                nc.vector.tensor_tensor(ab[:, 0:1], bc[:, 1:2],
                                        consts[:, COL_NWB + t:COL_NWB + t + 1],
                                        ALU.mult)
                tmp_b = scratch.tile([128, 1], F32, tag="gn_tmp")
                nc.vector.tensor_tensor(tmp_b[:], bc[:, 0:1], ab[:, 0:1],
                                        ALU.mult)
                nc.vector.tensor_tensor(
                    ab[:, 1:2],
                    consts[:, COL_NWB + 2 + t:COL_NWB + 3 + t], tmp_b[:],
                    ALU.subtract)
                ab_t.append(ab)

            xn_bf = persist.tile([128, 2, HW], BF16)
            for t in range(2):
                # xn = x*A + B, emitted bf16 (GpSimd: SBUF->SBUF is legal there)
                nc.gpsimd.tensor_scalar(xn_bf[:, t, :], x_sb[:, t, :],
                                        ab_t[t][:, 0:1], ab_t[t][:, 1:2],
                                        ALU.mult, ALU.add)

            # ---------------- qkv pass A: q,k [channel, hw] ----------------
            # psum tile m: 0,1 = q hg0/hg1 ; 2,3 = k hg0/hg1 (natural order)
            for m in (0, 2, 1, 3):
                ps = psump.tile([128, 2, 512], F32, tag="big", bufs=3)
                for n in range(2):
                    for kc in range(2):
                        nc.tensor.matmul(
                            ps[:, n, :],
                            wall[:, kc, 128 * m:128 * (m + 1)],
                            xn_bf[:, kc, 512 * n:512 * (n + 1)],
                            start=(kc == 0), stop=(kc == 1))
                dst = q8 if m < 2 else k8
                nc.scalar.activation(
                    dst[:, m % 2, 0, :], ps[:].rearrange("p n f -> p (n f)"),
                    AFT.Identity, bias=consts[:, COL_QKB + m:COL_QKB + m + 1])

            # ---------------- qkv pass B: v as [hw, channel] ---------------
            for cch in range(8):
                psb = psump.tile([128, 256], F32, tag="sm", bufs=2)
                for kc in range(2):
                    nc.tensor.matmul(
                        psb[:],
                        xn_bf[:, kc, 128 * cch:128 * (cch + 1)],
                        wall[:, kc, 512:768], start=(kc == 0), stop=(kc == 1))
                vv = vaug[:, :, cch, 0:32]
                pv = psb[:].rearrange("p (h d) -> p h d", d=32)
                if VEV_ASSIGN[cch] == 'A':
                    nc.scalar.copy(vv, pv)
                else:
                    nc.vector.tensor_copy(vv, pv)

            # ---------------- attention ----------------
            a_drams = [dramp.tile([32, HW], BF16, tag=f"adram{h}",
                                  name=f"a_dram{h}")
                       for h in range(N_HEADS)]
            a_sb = persist.tile([128, 2, HW], BF16)
            e_tiles = {}
            avp_tiles = {}

            def sim_exp_pair(hpair):
                for h in hpair:
                    e_tiles[h] = ering.tile([128, 8, HW], BF16, tag="e16",
                                            name=f"e16_{h}")
                    avp_tiles[h] = psump.tile([128, 8, 33], F32, tag="sm",
                                              bufs=2, name=f"avp_{h}")
                LAG = 2
                for jt in range(8):
                    for h in hpair:
                        sim_exp_one(h, jt)
                    if jt >= LAG:
                        for h in hpair:
                            av_jc(h, jt - LAG)
                for jc in range(8 - LAG, 8):
                    for h in hpair:
                        av_jc(h, jc)
                for h in hpair:
                    div_head(h)

            def sim_exp_one(h, jt):
                b_, hg = h % 4, h // 4
                e16 = e_tiles[h]
                if True:
                    sim = psump.tile([128, 2, 512], F32, tag="big", bufs=3,
                                     name=f"sim_{h}_{jt}")
                    for n in range(2):
                        nc.tensor.matmul(
                            sim[:, n, :],
                            k8[32 * b_:32 * b_ + 32, hg, :,
                               128 * jt:128 * (jt + 1)],
                            q8[32 * b_:32 * b_ + 32, hg, :,
                               512 * n:512 * (n + 1)],
                            start=True, stop=True, perf_mode=PM.DoubleRow,
                            tile_position=(32 * b_, 0))
                    eng = EXP_ASSIGN[h][jt]
                    simf = sim[:].rearrange("p n f -> p (n f)")
                    if eng == 'A':
                        nc.scalar.activation(
                            e16[:, jt, :], simf, AFT.Exp,
                            bias=consts[:, COL_ESH:COL_ESH + 1])
                    else:
                        nc.vector.tensor_scalar(
                            e16[:, jt, :].bitcast(I16), simf,
                            SCHR_SC, SCHR_C, ALU.mult, ALU.add)

            def av_jc(h, jc):
                e16 = e_tiles[h]
                avp = avp_tiles[h]
                for it in range(8):
                    nc.tensor.matmul(
                        avp[:, it, :],
                        e16[:, jc, 128 * it:128 * (it + 1)],
                        vaug[:, h, jc, :],
                        start=(jc == 0 and it == 0),
                        stop=(jc == 7 and it == 7),
                        skip_group_check=True)

            def div_head(h):
                avp = avp_tiles.pop(h)
                e_tiles.pop(h)
                recip = scratch.tile([128, 8], F32, tag="recip")
                nc.vector.reciprocal(recip[:], avp[:, :, 32])
                dv = scratch.tile([128, 8, 32], BF16, tag="avdiv", bufs=4,
                                  name=f"avdiv_{h}")
                rb = recip[:].unsqueeze(2).broadcast_to((128, 8, 32))
                nc.vector.tensor_tensor(dv[:], avp[:, :, 0:32], rb, ALU.mult)
                a_scat = a_drams[h][:].rearrange(
                    "(it g) (il d) -> g il it d", it=8, g=4, il=32)
                nc.sync.dma_start(a_scat, dv[:])
                nc.gpsimd.dma_start(
                    a_sb[32 * (h % 4):32 * (h % 4) + 32, h // 4, :],
                    a_drams[h][:])

            for p_ in range(4):
                sim_exp_pair((2 * p_, 2 * p_ + 1))

            # ---------------- out projection + residual ----------------
            for ot in range(2):
                y_sb = scratch.tile([128, HW], F32, tag="y_sb", bufs=2)
                for n in range(2):
                    ps = psump.tile([128, 512], F32, tag="sm", bufs=2,
                                    name=f"op_{ot}_{n}")
                    for ct in range(2):
                        nc.tensor.matmul(
                            ps[:],
                            wall[:, ct, 768 + 128 * ot:768 + 128 * (ot + 1)],
                            a_sb[:, ct, 512 * n:512 * (n + 1)],
                            start=(ct == 0), stop=False)
                    nc.tensor.matmul(
                        ps[:],
                        constsb[0:32, CB_YBT + 128 * ot:CB_YBT + 128 * (ot + 1)],
                        constsb[0:32, CB_P32 + 512 * n:CB_P32 + 512 * (n + 1)],
                        start=False, stop=True)
                    nc.vector.tensor_tensor(
                        y_sb[:, 512 * n:512 * (n + 1)], ps[:],
                        x_sb[:, ot, 512 * n:512 * (n + 1)], ALU.add)
                    dma_eng = nc.sync if ot == 0 else nc.scalar
                    dma_eng.dma_start(
                        y_out[128 * ot:128 * (ot + 1),
                              512 * n:512 * (n + 1)],
                        y_sb[:, 512 * n:512 * (n + 1)])

    ctx_lp.__exit__(None, None, None)
    nc.finalize()
    if fix_for_hw:
        nc.m = get_hw_module(nc.m)
        _split_excess_waits(nc.m)
    return nc


def host_prep(x, norm_w, norm_b, qkv_w, qkv_b, out_w, out_b):
    """Build per-core input maps from full inputs."""
    x = np.asarray(x, np.float32)
    qkv_w = np.asarray(qkv_w, np.float32)
    qkv_b = np.asarray(qkv_b, np.float32)
    out_w = np.asarray(out_w, np.float32)
    out_b = np.asarray(out_b, np.float32)
    norm_w = np.asarray(norm_w, np.float32)
    norm_b = np.asarray(norm_b, np.float32)

    wT = np.ascontiguousarray(qkv_w.T)          # [256, 768] in-ch major
    wqk = wT[:, 0:512].copy()
    wqk[:, 0:256] *= SCALE
    bqk = qkv_b[0:512].copy()
    bqk[0:256] *= SCALE
    wv = wT[:, 512:768]
    bv = qkv_b[512:768]
    woT = out_w.T                               # [256 in, 256 out]

    wall = np.zeros((128, 2, 1024), np.float32)
    for kc in range(2):
        wall[:, kc, 0:512] = wqk[128 * kc:128 * (kc + 1), :]
        wall[:, kc, 512:768] = wv[128 * kc:128 * (kc + 1), :]
        wall[:, kc, 768:1024] = woT[128 * kc:128 * (kc + 1), :]

    consts = np.zeros((128, CW), np.float32)
    consts[:, COL_NWB + 0] = norm_w[0:128]
    consts[:, COL_NWB + 1] = norm_w[128:256]
    consts[:, COL_NWB + 2] = norm_b[0:128]
    consts[:, COL_NWB + 3] = norm_b[128:256]
    p = np.arange(128)
    consts[p, COL_GIND + p // 8] = 1.0 / GROUP_SZ
    consts[p // 8, COL_GINDT + p] = 1.0  # rows 0:16
    for m in range(4):
        consts[:, COL_QKB + m] = bqk[128 * m:128 * (m + 1)]
    consts[:, COL_ESH] = ESHIFT
    consts[:, COL_EPS] = EPS

    # ybias[o, r] = sum_c Wo[o, c] * bv[(c//32)*32 + r] + out_b[o]
    bvpat = np.zeros((256, 32), np.float32)
    for c in range(256):
        bvpat[c, :] = bv[(c // 32) * 32 + np.arange(32)]
    ybias = out_w @ bvpat + out_b[:, None]      # [256, 32]

    constsb = np.zeros((128, CWB), np.float32)
    pp = np.arange(1024)
    constsb[pp % 32, CB_P32 + pp] = 1.0          # rows 0:32
    for ot in range(2):
        constsb[0:32, CB_YBT + 128 * ot:CB_YBT + 128 * (ot + 1)] = \
            ybias[128 * ot:128 * (ot + 1), :].T

    shared = {
        "wall": wall.astype(ml_dtypes.bfloat16),
        "consts": consts,
        "constsb": constsb.astype(ml_dtypes.bfloat16),
        "zeros8": np.zeros((128, 2, HW), ml_dtypes.float8_e4m3),
    }
    in_maps = []
    for b in range(N_CORES):
        m = dict(shared)
        m["x_in"] = np.ascontiguousarray(
            x[b].reshape(2, 128, HW).transpose(1, 0, 2))
        in_maps.append(m)
    return in_maps


_PROGRAM = None


def _get_program():
    global _PROGRAM
    if _PROGRAM is None:
        _PROGRAM = build_program()
    return _PROGRAM


def kernel(x, norm_w, norm_b, qkv_w, qkv_b, out_w, out_b, _trace=False):
    nc = _get_program()
    in_maps = host_prep(x, norm_w, norm_b, qkv_w, qkv_b, out_w, out_b)
    res = run_bass_kernel_spmd(nc, in_maps, list(range(N_CORES)), trace=_trace)
    out = np.stack([res.results[b]["y_out"].reshape(C, H, W)
                    for b in range(N_CORES)])
    if _trace:
        kernel.last_result = res
    return out.astype(np.float32)


# revision 19
# speedup vs baseline: 1.0758x; 1.0554x over previous
"""Trainium2 Bass kernel for nn_Attention_12970801234663.

Module: GroupNorm(32) -> 1x1 conv qkv -> 8-head attention over hw=1024 with the
original torch module's raw (b, heads, hw, head_dim) -> (b, c, h, w) reshape ->
1x1 out conv -> residual.

Sharding: pure data-parallel over batch (b=8) across 8 NeuronCores; weights are
broadcast. Each core computes one image end-to-end; no collectives.

Device-side plan (per core, c=256, hw=1024, heads=8, d=32), engineered against
the TimelineSim cost model (matmul cost = out-free-rows x cycles/row; fp8
DoubleRow = 0.5 cyc/row; ACT/DVE/Pool charge by free size):
  - GroupNorm stats via free-dim reduces + tiny PE matmuls against group
    indicator matrices; xn emitted in bf16 (tensor_scalar with per-channel
    A,B), which doubles as the qkv matmul operand conversion.
  - qkv projection in bf16. Pass A emits q,k with channels partition-packed
    as (32*(h%4)+d) so the fp8 DoubleRow sim can slice 32-aligned head
    blocks; eviction on ScalarE folds the qkv bias and converts to fp8e4
    into a zero-padded-double-row layout [128, hg, 2, 1024] (t=1 plane is
    zeros so DoubleRow's second k-tile contributes nothing).
  - sim[j,i] per (head, j-tile) via one fp8e4 DoubleRow matmul pair
    (N=512 each, 0.5 cyc/row) on 32-aligned partition blocks.
  - softmax exp with a constant -1.5 shift (cancels in the softmax ratio;
    keeps everything well inside bf16/fp8 range), statically split across
    three engines: ScalarE native Exp, and VectorE/GpSimd via a Schraudolph
    bf16 bit-trick (x*128/ln2 + 16251 -> int16 -> reinterpret bf16,
    ~2% rms).  All e tiles land in bf16.
  - attn@v in the transposed orientation out^T[i, (m|den)]: lhsT = e-chunk
    (stationary), rhs = [v^T | ones] (N=33) so output partitions are full
    (128 i's) and the softmax denominator rides along as one extra column.
  - softmax divide per head on VectorE/GpSimd, output bf16.
  - the module's scrambling reshape is a pure cross-partition collapse:
    one scatter DMA per head into a DRAM bounce in the scrambled channel
    order (64-byte runs), contiguous read-back per 128-channel tile.
  - out projection in bf16 with the (out_b + Wo@v-bias-pattern) term added
    via a rank-32 matmul against a (p%32) indicator, residual folded into
    the PSUM eviction op.
"""
import os
import sys

for p in ("/opt/trn_rl_repo",):
    if p not in sys.path and os.path.isdir(p):
        sys.path.insert(0, p)

import copy as _copy
import numpy as np
import ml_dtypes

import concourse.bass as bass
import concourse.tile as tile
from concourse import mybir
from concourse.bass_utils import run_bass_kernel_spmd
from concourse.bass_interp import get_hw_module

F32 = mybir.dt.float32
BF16 = mybir.dt.bfloat16
FP8E4 = mybir.dt.float8e4
I16 = mybir.dt.int16
ALU = mybir.AluOpType
AFT = mybir.ActivationFunctionType
PM = mybir.MatmulPerfMode

N_CORES = 8
B, C, H, W = 8, 256, 32, 32
HW = H * W                # 1024
N_HEADS = 8
HEAD_DIM = 32
GROUPS = 32
EPS = 1e-5
SCALE = HEAD_DIM ** -0.5
GROUP_SZ = (C // GROUPS) * HW  # 8192 elements per group

# softmax shift (cancels exactly in the softmax ratio)
ESHIFT = -1.5
# Schraudolph bf16 exp: bits = floor(x * 128/ln2 + 16251)
SCHR_SC = 128.0 / float(np.log(2.0))
SCHR_C = 16251.0 + ESHIFT * SCHR_SC

# fp32 consts columns
COL_NWB = 0      # 4: norm_w t0, norm_w t1, norm_b t0, norm_b t1
COL_GIND = 4     # 16: [128,16] group indicator
COL_GINDT = 20   # 128: rows 0:16 hold the [16,128] broadcast indicator
COL_QKB = 148    # 4: qk bias per pass-A psum tile (q0,q1,k0,k1), q scaled
COL_ESH = 152    # 1: ESHIFT broadcast column
COL_EPS = 153    # 1: GroupNorm eps broadcast column
CW = 154
# bf16 consts columns
CB_P32 = 0       # 1024: [32,1024] P32[r,p] = (p%32==r)
CB_YBT = 1024    # 256: [32, 2, 128] ybT[r, ot, o] = ybias[128*ot+o, r]
CWB = 1280

# exp engine assignment per (head, jtile): 'A'=ScalarE, 'D'=VectorE.
# (GPSIMD cannot touch PSUM, so it only gets SBUF-side work: xn + GN sums.)
EXP_ASSIGN = [['A', 'D', 'A', 'D', 'A', 'D', 'A', 'D'],
              ['D', 'A', 'D', 'A', 'D', 'A', 'D', 'A']] * 4
# v-eviction engine per hw-chunk
VEV_ASSIGN = ['A', 'D', 'A', 'D', 'A', 'D', 'A', 'D']
# divide engine per head
DIV_ASSIGN = ['D', 'D', 'D', 'D', 'D', 'D', 'D', 'D']


def _split_excess_waits(m):
    """Walrus in this toolchain accepts only one sem-wait per instruction;
    move excess waits onto preceding wait-only drains on the same engine."""
    n_split = 0
    for function in m.functions:
        new_blocks = []
        for block in function.blocks:
            new_insts = []
            for ins in block.instructions:
                si = ins.sync_info
                if si is None:
                    new_insts.append(ins)
                    continue
                waits = list(si.on_wait)
                if len(waits) > 1:
                    k = 0
                    while len(waits) > 1:
                        chunk, waits = waits[:1], waits[1:]
                        d = mybir.InstDrain(
                            name=f"{ins.name}-wsplit{k}",
                            ins=[], outs=[], bass_is_fusable=False,
                        )
                        d.engine = ins.engine
                        d.sync_info = mybir.SyncInfo(on_wait=chunk, on_update=[])
                        new_insts.append(d)
                        k += 1
                        n_split += 1
                    ins.sync_info = mybir.SyncInfo(
                        on_wait=waits, on_update=list(si.on_update))
                new_insts.append(ins)
            new_blocks.append(_copy.replace(block, instructions=new_insts))
        function.blocks.clear()
        function.blocks.extend(new_blocks)
    return n_split


def build_program(fix_for_hw=True):
    nc = bass.Bass("TRN2", target_bir_lowering=False, debug=False,
                   enable_asserts=False, num_devices=N_CORES)

    x_in = nc.dram_tensor("x_in", [128, 2, HW], F32, kind="ExternalInput")
    wall_in = nc.dram_tensor("wall", [128, 2, 1024], BF16, kind="ExternalInput")
    consts_in = nc.dram_tensor("consts", [128, CW], F32, kind="ExternalInput")
    constsb_in = nc.dram_tensor("constsb", [128, CWB], BF16,
                                kind="ExternalInput")
    zeros8_in = nc.dram_tensor("zeros8", [128, 2, HW], FP8E4,
                               kind="ExternalInput")
    y_out = nc.dram_tensor("y_out", [C, HW], F32, kind="ExternalOutput")

    ctx_lp = nc.allow_low_precision("bf16/fp8 attention by design")
    ctx_lp.__enter__()
    with tile.TileContext(nc) as tc:
        with (
            tc.tile_pool(name="persist", bufs=1) as persist,
            tc.tile_pool(name="ering", bufs=4) as ering,
            tc.tile_pool(name="scratch", bufs=2) as scratch,
            tc.tile_pool(name="psump", bufs=1, space="PSUM") as psump,
            tc.tile_pool(name="dramp", bufs=1, space="DRAM") as dramp,
        ):
            x_sb = persist.tile([128, 2, HW], F32)
            for t in range(2):
                nc.sync.dma_start(x_sb[:, t, :], x_in[:, t, :])
            consts = persist.tile([128, CW], F32)
            nc.sync.dma_start(consts[:], consts_in[:])
            # q/k fp8 zero-padded double-row layouts [128, hg, t, i]
            q8 = persist.tile([128, 2, 2, HW], FP8E4)
            k8 = persist.tile([128, 2, 2, HW], FP8E4)
            nc.sync.dma_start(q8[:, :, 1, :], zeros8_in[:])
            nc.sync.dma_start(k8[:, :, 1, :], zeros8_in[:])
            wall = persist.tile([128, 2, 1024], BF16)
            nc.sync.dma_start(wall[:], wall_in[:])
            constsb = persist.tile([128, CWB], BF16)
            nc.sync.dma_start(constsb[:], constsb_in[:])

            # [v^T | ones] per (head, jc) in bf16
            vaug = persist.tile([128, N_HEADS, 8, 33], BF16)
            nc.gpsimd.memset(vaug[:, :, :, 32:33], 1.0)

            # ---------------- GroupNorm ----------------
            ab_t = []
            for t in range(2):
                s_tile = scratch.tile([128, 2], F32, tag="gn_s")
                junk = scratch.tile([128, HW], F32, tag="junk")
                nc.scalar.activation(junk[:], x_sb[:, t, :], AFT.Copy,
                                     accum_out=s_tile[:, 0:1])
                junk2 = scratch.tile([128, HW], F32, tag="junk")
                nc.vector.scalar_tensor_tensor(
                    junk2[:], x_sb[:, t, :], 1.0, x_sb[:, t, :],
                    ALU.mult, ALU.mult, accum_out=s_tile[:, 1:2])
                gsum = psump.tile([16, 2], F32, tag="sm", bufs=2)
                nc.tensor.matmul(gsum[:], consts[:, COL_GIND:COL_GIND + 16],
                                 s_tile[:])
                mu_rs = scratch.tile([16, 2], F32, tag="gn_mr")
                nc.vector.tensor_copy(mu_rs[:, 0:1], gsum[:, 0:1])
                var_t = scratch.tile([16, 1], F32, tag="gn_var")
                nc.vector.tensor_tensor(var_t[:], mu_rs[:, 0:1],
                                        mu_rs[:, 0:1], ALU.mult)
                nc.vector.tensor_tensor(var_t[:], gsum[:, 1:2], var_t[:],
                                        ALU.subtract)
                ln_t = scratch.tile([16, 1], F32, tag="gn_ln")
                nc.scalar.activation(ln_t[:], var_t[:], AFT.Ln,
                                     bias=consts[0:16, COL_EPS:COL_EPS + 1])
                nc.scalar.activation(mu_rs[:, 1:2], ln_t[:], AFT.Exp,
                                     scale=-0.5)
                bc = psump.tile([128, 2], F32, tag="sm", bufs=2)
                nc.tensor.matmul(bc[:], consts[0:16, COL_GINDT:COL_GINDT + 128],
                                 mu_rs[:])
                ab = scratch.tile([128, 2], F32, tag="gn_ab", bufs=2)
                # A = rsqrt * w ; B = b - mu * A
                nc.vector.tensor_tensor(ab[:, 0:1], bc[:, 1:2],
                                        consts[:, COL_NWB + t:COL_NWB + t + 1],
                                        ALU.mult)
                tmp_b = scratch.tile([128, 1], F32, tag="gn_tmp")
                nc.vector.tensor_tensor(tmp_b[:], bc[:, 0:1], ab[:, 0:1],
                                        ALU.mult)
                nc.vector.tensor_tensor(
                    ab[:, 1:2],
                    consts[:, COL_NWB + 2 + t:COL_NWB + 3 + t], tmp_b[:],
                    ALU.subtract)
                ab_t.append(ab)

            xn_bf = persist.tile([128, 2, HW], BF16)
            for t in range(2):
                # xn = x*A + B, emitted bf16 (GpSimd: SBUF->SBUF is legal there)
                nc.vector.tensor_scalar(xn_bf[:, t, :], x_sb[:, t, :],
                                        ab_t[t][:, 0:1], ab_t[t][:, 1:2],
                                        ALU.mult, ALU.add)

            # ---------------- qkv pass A: q,k [channel, hw] ----------------
            # psum tile m: 0,1 = q hg0/hg1 ; 2,3 = k hg0/hg1 (natural order)
            for m in (0, 2, 1, 3):
                ps = psump.tile([128, 2, 512], F32, tag="big", bufs=3)
                for n in range(2):
                    for kc in range(2):
                        nc.tensor.matmul(
                            ps[:, n, :],
                            wall[:, kc, 128 * m:128 * (m + 1)],
                            xn_bf[:, kc, 512 * n:512 * (n + 1)],
                            start=(kc == 0), stop=(kc == 1))
                dst = q8 if m < 2 else k8
                psf = ps[:].rearrange("p n f -> p (n f)")
                if m < 2:
                    nc.scalar.activation(
                        dst[:, m % 2, 0, :], psf, AFT.Identity,
                        bias=consts[:, COL_QKB + m:COL_QKB + m + 1])
                else:
                    nc.vector.tensor_scalar(
                        dst[:, m % 2, 0, :], psf, 1.0,
                        consts[:, COL_QKB + m:COL_QKB + m + 1],
                        ALU.mult, ALU.add)

            # ---------------- qkv pass B: v as [hw, channel] ---------------
            for cch in range(8):
                psb = psump.tile([128, 256], F32, tag="sm", bufs=2)
                for kc in range(2):
                    nc.tensor.matmul(
                        psb[:],
                        xn_bf[:, kc, 128 * cch:128 * (cch + 1)],
                        wall[:, kc, 512:768], start=(kc == 0), stop=(kc == 1))
                vv = vaug[:, :, cch, 0:32]
                pv = psb[:].rearrange("p (h d) -> p h d", d=32)
                if VEV_ASSIGN[cch] == 'A':
                    nc.scalar.copy(vv, pv)
                else:
                    nc.vector.tensor_copy(vv, pv)

            # ---------------- attention ----------------
            a_drams = [dramp.tile([32, HW], BF16, tag=f"adram{h}",
                                  name=f"a_dram{h}")
                       for h in range(N_HEADS)]
            a_sb = persist.tile([128, 2, HW], BF16)
            e_tiles = {}
            avp_tiles = {}

            def sim_exp_pair(hpair):
                for h in hpair:
                    e_tiles[h] = ering.tile([128, 8, HW], BF16, tag="e16",
                                            name=f"e16_{h}")
                    avp_tiles[h] = psump.tile([128, 8, 33], F32, tag="sm",
                                              bufs=2, name=f"avp_{h}")
                LAG = 2
                for jt in range(8):
                    for h in hpair:
                        sim_exp_one(h, jt)
                    if jt >= LAG:
                        for h in hpair:
                            av_jc(h, jt - LAG)
                for jc in range(8 - LAG, 8):
                    for h in hpair:
                        av_jc(h, jc)
                for h in hpair:
                    div_head(h)

            def sim_exp_one(h, jt):
                b_, hg = h % 4, h // 4
                e16 = e_tiles[h]
                if True:
                    sim = psump.tile([128, 2, 512], F32, tag="big", bufs=3,
                                     name=f"sim_{h}_{jt}")
                    for n in range(2):
                        nc.tensor.matmul(
                            sim[:, n, :],
                            k8[32 * b_:32 * b_ + 32, hg, :,
                               128 * jt:128 * (jt + 1)],
                            q8[32 * b_:32 * b_ + 32, hg, :,
                               512 * n:512 * (n + 1)],
                            start=True, stop=True, perf_mode=PM.DoubleRow,
                            tile_position=(32 * b_, 0))
                    eng = EXP_ASSIGN[h][jt]
                    simf = sim[:].rearrange("p n f -> p (n f)")
                    if eng == 'A':
                        nc.scalar.activation(
                            e16[:, jt, :], simf, AFT.Exp,
                            bias=consts[:, COL_ESH:COL_ESH + 1])
                    else:
                        nc.vector.tensor_scalar(
                            e16[:, jt, :].bitcast(I16), simf,
                            SCHR_SC, SCHR_C, ALU.mult, ALU.add)

            def av_jc(h, jc):
                e16 = e_tiles[h]
                avp = avp_tiles[h]
                for it in range(8):
                    nc.tensor.matmul(
                        avp[:, it, :],
                        e16[:, jc, 128 * it:128 * (it + 1)],
                        vaug[:, h, jc, :],
                        start=(jc == 0 and it == 0),
                        stop=(jc == 7 and it == 7),
                        skip_group_check=True)

            def div_head(h):
                avp = avp_tiles.pop(h)
                e_tiles.pop(h)
                recip = scratch.tile([128, 8], F32, tag="recip")
                nc.vector.reciprocal(recip[:], avp[:, :, 32])
                dv = scratch.tile([128, 8, 32], BF16, tag="avdiv", bufs=4,
                                  name=f"avdiv_{h}")
                rb = recip[:].unsqueeze(2).broadcast_to((128, 8, 32))
                nc.vector.tensor_tensor(dv[:], avp[:, :, 0:32], rb, ALU.mult)
                a_scat = a_drams[h][:].rearrange(
                    "(it g) (il d) -> g il it d", it=8, g=4, il=32)
                nc.sync.dma_start(a_scat, dv[:])
                nc.gpsimd.dma_start(
                    a_sb[32 * (h % 4):32 * (h % 4) + 32, h // 4, :],
                    a_drams[h][:])

            for p_ in range(4):
                sim_exp_pair((2 * p_, 2 * p_ + 1))

            # ---------------- out projection + residual ----------------
            # PE warm-keepers across the readback latency window (the p-state
            # model halves matmul speed after a >100ns idle gap)
            warm = psump.tile([128, 512], F32, tag="big", bufs=3)
            for w in range(10):
                nc.tensor.matmul(
                    warm[:], constsb[0:32, CB_YBT:CB_YBT + 128],
                    constsb[0:32, CB_P32:CB_P32 + 512],
                    start=(w == 0), stop=(w == 9))
            op_ps = {}
            for gi, (ot, n) in enumerate(
                    ((0, 0), (0, 1), (1, 0), (1, 1))):
                tag = "big" if gi < 2 else "sm"
                ps = psump.tile([128, 512], F32, tag=tag,
                                bufs=3 if gi < 2 else 2,
                                name=f"op_{ot}_{n}")
                op_ps[(ot, n)] = ps
                nc.tensor.matmul(
                    ps[:],
                    constsb[0:32, CB_YBT + 128 * ot:CB_YBT + 128 * (ot + 1)],
                    constsb[0:32, CB_P32 + 512 * n:CB_P32 + 512 * (n + 1)],
                    start=True, stop=False)
                nc.tensor.matmul(
                    ps[:],
                    wall[:, 0, 768 + 128 * ot:768 + 128 * (ot + 1)],
                    a_sb[:, 0, 512 * n:512 * (n + 1)],
                    start=False, stop=False)
            y_sbs = {}
            for ot in range(2):
                y_sbs[ot] = scratch.tile([128, HW], F32, tag="y_sb", bufs=2,
                                         name=f"y_sb{ot}")
            for ot in range(2):
                for n in range(2):
                    ps = op_ps[(ot, n)]
                    nc.tensor.matmul(
                        ps[:],
                        wall[:, 1, 768 + 128 * ot:768 + 128 * (ot + 1)],
                        a_sb[:, 1, 512 * n:512 * (n + 1)],
                        start=False, stop=True)
                    y_sb = y_sbs[ot]
                    nc.vector.tensor_tensor(
                        y_sb[:, 512 * n:512 * (n + 1)], ps[:],
                        x_sb[:, ot, 512 * n:512 * (n + 1)], ALU.add)
                    dma_eng = nc.sync if ot == 0 else nc.scalar
                    dma_eng.dma_start(
                        y_out[128 * ot:128 * (ot + 1),
                              512 * n:512 * (n + 1)],
                        y_sb[:, 512 * n:512 * (n + 1)])

    ctx_lp.__exit__(None, None, None)
    nc.finalize()
    if fix_for_hw:
        nc.m = get_hw_module(nc.m)
        _split_excess_waits(nc.m)
    return nc


def host_prep(x, norm_w, norm_b, qkv_w, qkv_b, out_w, out_b):
    """Build per-core input maps from full inputs."""
    x = np.asarray(x, np.float32)
    qkv_w = np.asarray(qkv_w, np.float32)
    qkv_b = np.asarray(qkv_b, np.float32)
    out_w = np.asarray(out_w, np.float32)
    out_b = np.asarray(out_b, np.float32)
    norm_w = np.asarray(norm_w, np.float32)
    norm_b = np.asarray(norm_b, np.float32)

    wT = np.ascontiguousarray(qkv_w.T)          # [256, 768] in-ch major
    wqk = wT[:, 0:512].copy()
    wqk[:, 0:256] *= SCALE
    bqk = qkv_b[0:512].copy()
    bqk[0:256] *= SCALE
    wv = wT[:, 512:768]
    bv = qkv_b[512:768]
    woT = out_w.T                               # [256 in, 256 out]

    wall = np.zeros((128, 2, 1024), np.float32)
    for kc in range(2):
        wall[:, kc, 0:512] = wqk[128 * kc:128 * (kc + 1), :]
        wall[:, kc, 512:768] = wv[128 * kc:128 * (kc + 1), :]
        wall[:, kc, 768:1024] = woT[128 * kc:128 * (kc + 1), :]

    consts = np.zeros((128, CW), np.float32)
    consts[:, COL_NWB + 0] = norm_w[0:128]
    consts[:, COL_NWB + 1] = norm_w[128:256]
    consts[:, COL_NWB + 2] = norm_b[0:128]
    consts[:, COL_NWB + 3] = norm_b[128:256]
    p = np.arange(128)
    consts[p, COL_GIND + p // 8] = 1.0 / GROUP_SZ
    consts[p // 8, COL_GINDT + p] = 1.0  # rows 0:16
    for m in range(4):
        consts[:, COL_QKB + m] = bqk[128 * m:128 * (m + 1)]
    consts[:, COL_ESH] = ESHIFT
    consts[:, COL_EPS] = EPS

    # ybias[o, r] = sum_c Wo[o, c] * bv[(c//32)*32 + r] + out_b[o]
    bvpat = np.zeros((256, 32), np.float32)
    for c in range(256):
        bvpat[c, :] = bv[(c // 32) * 32 + np.arange(32)]
    ybias = out_w @ bvpat + out_b[:, None]      # [256, 32]

    constsb = np.zeros((128, CWB), np.float32)
    pp = np.arange(1024)
    constsb[pp % 32, CB_P32 + pp] = 1.0          # rows 0:32
    for ot in range(2):
        constsb[0:32, CB_YBT + 128 * ot:CB_YBT + 128 * (ot + 1)] = \
            ybias[128 * ot:128 * (ot + 1), :].T

    shared = {
        "wall": wall.astype(ml_dtypes.bfloat16),
        "consts": consts,
        "constsb": constsb.astype(ml_dtypes.bfloat16),
        "zeros8": np.zeros((128, 2, HW), ml_dtypes.float8_e4m3),
    }
    in_maps = []
    for b in range(N_CORES):
        m = dict(shared)
        m["x_in"] = np.ascontiguousarray(
            x[b].reshape(2, 128, HW).transpose(1, 0, 2))
        in_maps.append(m)
    return in_maps


_PROGRAM = None


def _get_program():
    global _PROGRAM
    if _PROGRAM is None:
        _PROGRAM = build_program()
    return _PROGRAM


def kernel(x, norm_w, norm_b, qkv_w, qkv_b, out_w, out_b, _trace=False):
    nc = _get_program()
    in_maps = host_prep(x, norm_w, norm_b, qkv_w, qkv_b, out_w, out_b)
    res = run_bass_kernel_spmd(nc, in_maps, list(range(N_CORES)), trace=_trace)
    out = np.stack([res.results[b]["y_out"].reshape(C, H, W)
                    for b in range(N_CORES)])
    if _trace:
        kernel.last_result = res
    return out.astype(np.float32)


# revision 20
# speedup vs baseline: 1.0866x; 1.0100x over previous
"""Trainium2 Bass kernel for nn_Attention_12970801234663.

Module: GroupNorm(32) -> 1x1 conv qkv -> 8-head attention over hw=1024 with the
original torch module's raw (b, heads, hw, head_dim) -> (b, c, h, w) reshape ->
1x1 out conv -> residual.

Sharding: pure data-parallel over batch (b=8) across 8 NeuronCores; weights are
broadcast. Each core computes one image end-to-end; no collectives.

Device-side plan (per core, c=256, hw=1024, heads=8, d=32), engineered against
the TimelineSim cost model (matmul cost = out-free-rows x cycles/row; fp8
DoubleRow = 0.5 cyc/row; ACT/DVE/Pool charge by free size):
  - GroupNorm stats via free-dim reduces + tiny PE matmuls against group
    indicator matrices; xn emitted in bf16 (tensor_scalar with per-channel
    A,B), which doubles as the qkv matmul operand conversion.
  - qkv projection in bf16. Pass A emits q,k with channels partition-packed
    as (32*(h%4)+d) so the fp8 DoubleRow sim can slice 32-aligned head
    blocks; eviction on ScalarE folds the qkv bias and converts to fp8e4
    into a zero-padded-double-row layout [128, hg, 2, 1024] (t=1 plane is
    zeros so DoubleRow's second k-tile contributes nothing).
  - sim[j,i] per (head, j-tile) via one fp8e4 DoubleRow matmul pair
    (N=512 each, 0.5 cyc/row) on 32-aligned partition blocks.
  - softmax exp with a constant -1.5 shift (cancels in the softmax ratio;
    keeps everything well inside bf16/fp8 range), statically split across
    three engines: ScalarE native Exp, and VectorE/GpSimd via a Schraudolph
    bf16 bit-trick (x*128/ln2 + 16251 -> int16 -> reinterpret bf16,
    ~2% rms).  All e tiles land in bf16.
  - attn@v in the transposed orientation out^T[i, (m|den)]: lhsT = e-chunk
    (stationary), rhs = [v^T | ones] (N=33) so output partitions are full
    (128 i's) and the softmax denominator rides along as one extra column.
  - softmax divide per head on VectorE/GpSimd, output bf16.
  - the module's scrambling reshape is a pure cross-partition collapse:
    one scatter DMA per head into a DRAM bounce in the scrambled channel
    order (64-byte runs), contiguous read-back per 128-channel tile.
  - out projection in bf16 with the (out_b + Wo@v-bias-pattern) term added
    via a rank-32 matmul against a (p%32) indicator, residual folded into
    the PSUM eviction op.
"""
import os
import sys

for p in ("/opt/trn_rl_repo",):
    if p not in sys.path and os.path.isdir(p):
        sys.path.insert(0, p)

import copy as _copy
import numpy as np
import ml_dtypes

import concourse.bass as bass
import concourse.tile as tile
from concourse import mybir
from concourse.bass_utils import run_bass_kernel_spmd
from concourse.bass_interp import get_hw_module

F32 = mybir.dt.float32
BF16 = mybir.dt.bfloat16
FP8E4 = mybir.dt.float8e4
I16 = mybir.dt.int16
ALU = mybir.AluOpType
AFT = mybir.ActivationFunctionType
PM = mybir.MatmulPerfMode

N_CORES = 8
B, C, H, W = 8, 256, 32, 32
HW = H * W                # 1024
N_HEADS = 8
HEAD_DIM = 32
GROUPS = 32
EPS = 1e-5
SCALE = HEAD_DIM ** -0.5
GROUP_SZ = (C // GROUPS) * HW  # 8192 elements per group

# softmax shift (cancels exactly in the softmax ratio)
ESHIFT = -1.5
# Schraudolph bf16 exp: bits = floor(x * 128/ln2 + 16251)
SCHR_SC = 128.0 / float(np.log(2.0))
SCHR_C = 16251.0 + ESHIFT * SCHR_SC

# fp32 consts columns
COL_NWB = 0      # 4: norm_w t0, norm_w t1, norm_b t0, norm_b t1
COL_GIND = 4     # 16: [128,16] group indicator
COL_GINDT = 20   # 128: rows 0:16 hold the [16,128] broadcast indicator
COL_QKB = 148    # 4: qk bias per pass-A psum tile (q0,q1,k0,k1), q scaled
COL_ESH = 152    # 1: ESHIFT broadcast column
COL_EPS = 153    # 1: GroupNorm eps broadcast column
CW = 154
# bf16 consts columns
CB_P32 = 0       # 1024: [32,1024] P32[r,p] = (p%32==r)
CB_YBT = 1024    # 256: [32, 2, 128] ybT[r, ot, o] = ybias[128*ot+o, r]
CWB = 1280

# exp engine assignment per (head, jtile): 'A'=ScalarE, 'D'=VectorE.
# (GPSIMD cannot touch PSUM, so it only gets SBUF-side work: xn + GN sums.)
EXP_ASSIGN = [['A', 'D', 'A', 'D', 'A', 'A', 'D', 'A'],
              ['D', 'A', 'D', 'A', 'A', 'D', 'A', 'D']] * 4
# v-eviction engine per hw-chunk
VEV_ASSIGN = ['A', 'D', 'A', 'D', 'A', 'D', 'A', 'D']
# divide engine per head
DIV_ASSIGN = ['D', 'D', 'D', 'D', 'D', 'D', 'D', 'D']


def _split_excess_waits(m):
    """Walrus in this toolchain accepts only one sem-wait per instruction;
    move excess waits onto preceding wait-only drains on the same engine."""
    n_split = 0
    for function in m.functions:
        new_blocks = []
        for block in function.blocks:
            new_insts = []
            for ins in block.instructions:
                si = ins.sync_info
                if si is None:
                    new_insts.append(ins)
                    continue
                waits = list(si.on_wait)
                if len(waits) > 1:
                    k = 0
                    while len(waits) > 1:
                        chunk, waits = waits[:1], waits[1:]
                        d = mybir.InstDrain(
                            name=f"{ins.name}-wsplit{k}",
                            ins=[], outs=[], bass_is_fusable=False,
                        )
                        d.engine = ins.engine
                        d.sync_info = mybir.SyncInfo(on_wait=chunk, on_update=[])
                        new_insts.append(d)
                        k += 1
                        n_split += 1
                    ins.sync_info = mybir.SyncInfo(
                        on_wait=waits, on_update=list(si.on_update))
                new_insts.append(ins)
            new_blocks.append(_copy.replace(block, instructions=new_insts))
        function.blocks.clear()
        function.blocks.extend(new_blocks)
    return n_split


def build_program(fix_for_hw=True):
    nc = bass.Bass("TRN2", target_bir_lowering=False, debug=False,
                   enable_asserts=False, num_devices=N_CORES)

    x_in = nc.dram_tensor("x_in", [128, 2, HW], F32, kind="ExternalInput")
    wall_in = nc.dram_tensor("wall", [128, 2, 1024], BF16, kind="ExternalInput")
    consts_in = nc.dram_tensor("consts", [128, CW], F32, kind="ExternalInput")
    constsb_in = nc.dram_tensor("constsb", [128, CWB], BF16,
                                kind="ExternalInput")
    zeros8_in = nc.dram_tensor("zeros8", [128, 2, HW], FP8E4,
                               kind="ExternalInput")
    y_out = nc.dram_tensor("y_out", [C, HW], F32, kind="ExternalOutput")

    ctx_lp = nc.allow_low_precision("bf16/fp8 attention by design")
    ctx_lp.__enter__()
    with tile.TileContext(nc) as tc:
        with (
            tc.tile_pool(name="persist", bufs=1) as persist,
            tc.tile_pool(name="ering", bufs=4) as ering,
            tc.tile_pool(name="scratch", bufs=2) as scratch,
            tc.tile_pool(name="psump", bufs=1, space="PSUM") as psump,
            tc.tile_pool(name="dramp", bufs=1, space="DRAM") as dramp,
        ):
            x_sb = persist.tile([128, 2, HW], F32)
            for t in range(2):
                nc.sync.dma_start(x_sb[:, t, :], x_in[:, t, :])
            consts = persist.tile([128, CW], F32)
            nc.sync.dma_start(consts[:], consts_in[:])
            # q/k fp8 zero-padded double-row layouts [128, hg, t, i]
            q8 = persist.tile([128, 2, 2, HW], FP8E4)
            k8 = persist.tile([128, 2, 2, HW], FP8E4)
            nc.sync.dma_start(q8[:, :, 1, :], zeros8_in[:])
            nc.sync.dma_start(k8[:, :, 1, :], zeros8_in[:])
            wall = persist.tile([128, 2, 1024], BF16)
            nc.sync.dma_start(wall[:], wall_in[:])
            constsb = persist.tile([128, CWB], BF16)
            nc.sync.dma_start(constsb[:], constsb_in[:])

            # [v^T | ones] per (head, jc) in bf16
            vaug = persist.tile([128, N_HEADS, 8, 33], BF16)
            nc.gpsimd.memset(vaug[:, :, :, 32:33], 1.0)

            # ---------------- GroupNorm ----------------
            ab_t = []
            for t in range(2):
                s_tile = scratch.tile([128, 2], F32, tag="gn_s")
                junk = scratch.tile([128, HW], F32, tag="junk")
                nc.scalar.activation(junk[:], x_sb[:, t, :], AFT.Copy,
                                     accum_out=s_tile[:, 0:1])
                junk2 = scratch.tile([128, HW], F32, tag="junk")
                nc.vector.scalar_tensor_tensor(
                    junk2[:], x_sb[:, t, :], 1.0, x_sb[:, t, :],
                    ALU.mult, ALU.mult, accum_out=s_tile[:, 1:2])
                gsum = psump.tile([16, 2], F32, tag="sm", bufs=2)
                nc.tensor.matmul(gsum[:], consts[:, COL_GIND:COL_GIND + 16],
                                 s_tile[:])
                mu_rs = scratch.tile([16, 2], F32, tag="gn_mr")
                nc.vector.tensor_copy(mu_rs[:, 0:1], gsum[:, 0:1])
                var_t = scratch.tile([16, 1], F32, tag="gn_var")
                nc.vector.tensor_tensor(var_t[:], mu_rs[:, 0:1],
                                        mu_rs[:, 0:1], ALU.mult)
                nc.vector.tensor_tensor(var_t[:], gsum[:, 1:2], var_t[:],
                                        ALU.subtract)
                ln_t = scratch.tile([16, 1], F32, tag="gn_ln")
                nc.scalar.activation(ln_t[:], var_t[:], AFT.Ln,
                                     bias=consts[0:16, COL_EPS:COL_EPS + 1])
                nc.scalar.activation(mu_rs[:, 1:2], ln_t[:], AFT.Exp,
                                     scale=-0.5)
                bc = psump.tile([128, 2], F32, tag="sm", bufs=2)
                nc.tensor.matmul(bc[:], consts[0:16, COL_GINDT:COL_GINDT + 128],
                                 mu_rs[:])
                ab = scratch.tile([128, 2], F32, tag="gn_ab", bufs=2)
                # A = rsqrt * w ; B = b - mu * A
                nc.vector.tensor_tensor(ab[:, 0:1], bc[:, 1:2],
                                        consts[:, COL_NWB + t:COL_NWB + t + 1],
                                        ALU.mult)
                tmp_b = scratch.tile([128, 1], F32, tag="gn_tmp")
                nc.vector.tensor_tensor(tmp_b[:], bc[:, 0:1], ab[:, 0:1],
                                        ALU.mult)
                nc.vector.tensor_tensor(
                    ab[:, 1:2],
                    consts[:, COL_NWB + 2 + t:COL_NWB + 3 + t], tmp_b[:],
                    ALU.subtract)
                ab_t.append(ab)

            xn_bf = persist.tile([128, 2, HW], BF16)
            for t in range(2):
                # xn = x*A + B, emitted bf16 (GpSimd: SBUF->SBUF is legal there)
                nc.vector.tensor_scalar(xn_bf[:, t, :], x_sb[:, t, :],
                                        ab_t[t][:, 0:1], ab_t[t][:, 1:2],
                                        ALU.mult, ALU.add)

            # ---------------- qkv pass A: q,k [channel, hw] ----------------
            # psum tile m: 0,1 = q hg0/hg1 ; 2,3 = k hg0/hg1 (natural order)
            for m in (0, 2, 1, 3):
                ps = psump.tile([128, 2, 512], F32, tag="big", bufs=3)
                for n in range(2):
                    for kc in range(2):
                        nc.tensor.matmul(
                            ps[:, n, :],
                            wall[:, kc, 128 * m:128 * (m + 1)],
                            xn_bf[:, kc, 512 * n:512 * (n + 1)],
                            start=(kc == 0), stop=(kc == 1))
                dst = q8 if m < 2 else k8
                psf = ps[:].rearrange("p n f -> p (n f)")
                if m < 2:
                    nc.scalar.activation(
                        dst[:, m % 2, 0, :], psf, AFT.Identity,
                        bias=consts[:, COL_QKB + m:COL_QKB + m + 1])
                else:
                    nc.vector.tensor_scalar(
                        dst[:, m % 2, 0, :], psf, 1.0,
                        consts[:, COL_QKB + m:COL_QKB + m + 1],
                        ALU.mult, ALU.add)

            # ---------------- qkv pass B: v as [hw, channel] ---------------
            for cch in range(8):
                psb = psump.tile([128, 256], F32, tag="sm", bufs=2)
                for kc in range(2):
                    nc.tensor.matmul(
                        psb[:],
                        xn_bf[:, kc, 128 * cch:128 * (cch + 1)],
                        wall[:, kc, 512:768], start=(kc == 0), stop=(kc == 1))
                vv = vaug[:, :, cch, 0:32]
                pv = psb[:].rearrange("p (h d) -> p h d", d=32)
                if VEV_ASSIGN[cch] == 'A':
                    nc.scalar.copy(vv, pv)
                else:
                    nc.vector.tensor_copy(vv, pv)

            # ---------------- attention ----------------
            a_drams = [dramp.tile([32, HW], BF16, tag=f"adram{h}",
                                  name=f"a_dram{h}")
                       for h in range(N_HEADS)]
            a_sb = persist.tile([128, 2, HW], BF16)
            e_tiles = {}
            avp_tiles = {}

            def sim_exp_pair(hpair):
                for h in hpair:
                    e_tiles[h] = ering.tile([128, 8, HW], BF16, tag="e16",
                                            name=f"e16_{h}")
                    avp_tiles[h] = psump.tile([128, 8, 33], F32, tag="sm",
                                              bufs=2, name=f"avp_{h}")
                LAG = 2
                for jt in range(8):
                    for h in hpair:
                        sim_exp_one(h, jt)
                    if jt >= LAG:
                        for h in hpair:
                            av_jc(h, jt - LAG)
                for jc in range(8 - LAG, 8):
                    for h in hpair:
                        av_jc(h, jc)
                for h in hpair:
                    div_head(h)

            def sim_exp_one(h, jt):
                b_, hg = h % 4, h // 4
                e16 = e_tiles[h]
                if True:
                    sim = psump.tile([128, 2, 512], F32, tag="big", bufs=3,
                                     name=f"sim_{h}_{jt}")
                    for n in range(2):
                        nc.tensor.matmul(
                            sim[:, n, :],
                            k8[32 * b_:32 * b_ + 32, hg, :,
                               128 * jt:128 * (jt + 1)],
                            q8[32 * b_:32 * b_ + 32, hg, :,
                               512 * n:512 * (n + 1)],
                            start=True, stop=True, perf_mode=PM.DoubleRow,
                            tile_position=(32 * b_, 0))
                    eng = EXP_ASSIGN[h][jt]
                    simf = sim[:].rearrange("p n f -> p (n f)")
                    if eng == 'A':
                        nc.scalar.activation(
                            e16[:, jt, :], simf, AFT.Exp,
                            bias=consts[:, COL_ESH:COL_ESH + 1])
                    else:
                        nc.vector.tensor_scalar(
                            e16[:, jt, :].bitcast(I16), simf,
                            SCHR_SC, SCHR_C, ALU.mult, ALU.add)

            def av_jc(h, jc):
                e16 = e_tiles[h]
                avp = avp_tiles[h]
                for it in range(8):
                    nc.tensor.matmul(
                        avp[:, it, :],
                        e16[:, jc, 128 * it:128 * (it + 1)],
                        vaug[:, h, jc, :],
                        start=(jc == 0 and it == 0),
                        stop=(jc == 7 and it == 7),
                        skip_group_check=True)

            def div_head(h):
                avp = avp_tiles.pop(h)
                e_tiles.pop(h)
                recip = scratch.tile([128, 8], F32, tag="recip")
                nc.vector.reciprocal(recip[:], avp[:, :, 32])
                dv = scratch.tile([128, 8, 32], BF16, tag="avdiv", bufs=4,
                                  name=f"avdiv_{h}")
                rb = recip[:].unsqueeze(2).broadcast_to((128, 8, 32))
                nc.vector.tensor_tensor(dv[:], avp[:, :, 0:32], rb, ALU.mult)
                a_scat = a_drams[h][:].rearrange(
                    "(it g) (il d) -> g il it d", it=8, g=4, il=32)
                nc.sync.dma_start(a_scat, dv[:])
                nc.gpsimd.dma_start(
                    a_sb[32 * (h % 4):32 * (h % 4) + 32, h // 4, :],
                    a_drams[h][:])

            for p_ in range(4):
                sim_exp_pair((2 * p_, 2 * p_ + 1))

            # ---------------- out projection + residual ----------------
            # PE warm-keepers across the readback latency window (the p-state
            # model halves matmul speed after a >100ns idle gap)
            warm = psump.tile([128, 512], F32, tag="big", bufs=3)
            for w in range(10):
                nc.tensor.matmul(
                    warm[:], constsb[0:32, CB_YBT:CB_YBT + 128],
                    constsb[0:32, CB_P32:CB_P32 + 512],
                    start=(w == 0), stop=(w == 9))
            op_ps = {}
            for gi, (ot, n) in enumerate(
                    ((0, 0), (0, 1), (1, 0), (1, 1))):
                tag = "big" if gi < 2 else "sm"
                ps = psump.tile([128, 512], F32, tag=tag,
                                bufs=3 if gi < 2 else 2,
                                name=f"op_{ot}_{n}")
                op_ps[(ot, n)] = ps
                nc.tensor.matmul(
                    ps[:],
                    constsb[0:32, CB_YBT + 128 * ot:CB_YBT + 128 * (ot + 1)],
                    constsb[0:32, CB_P32 + 512 * n:CB_P32 + 512 * (n + 1)],
                    start=True, stop=False)
                nc.tensor.matmul(
                    ps[:],
                    wall[:, 0, 768 + 128 * ot:768 + 128 * (ot + 1)],
                    a_sb[:, 0, 512 * n:512 * (n + 1)],
                    start=False, stop=False)
            y_sbs = {}
            for ot in range(2):
                y_sbs[ot] = scratch.tile([128, HW], F32, tag="y_sb", bufs=2,
                                         name=f"y_sb{ot}")
            for ot in range(2):
                for n in range(2):
                    ps = op_ps[(ot, n)]
                    nc.tensor.matmul(
                        ps[:],
                        wall[:, 1, 768 + 128 * ot:768 + 128 * (ot + 1)],
                        a_sb[:, 1, 512 * n:512 * (n + 1)],
                        start=False, stop=True)
                    y_sb = y_sbs[ot]
                    nc.vector.tensor_tensor(
                        y_sb[:, 512 * n:512 * (n + 1)], ps[:],
                        x_sb[:, ot, 512 * n:512 * (n + 1)], ALU.add)
                    dma_eng = nc.sync if ot == 0 else nc.scalar
                    dma_eng.dma_start(
                        y_out[128 * ot:128 * (ot + 1),
                              512 * n:512 * (n + 1)],
                        y_sb[:, 512 * n:512 * (n + 1)])

    ctx_lp.__exit__(None, None, None)
    nc.finalize()
    if fix_for_hw:
        nc.m = get_hw_module(nc.m)
        _split_excess_waits(nc.m)
    return nc


def host_prep(x, norm_w, norm_b, qkv_w, qkv_b, out_w, out_b):
    """Build per-core input maps from full inputs."""
    x = np.asarray(x, np.float32)
    qkv_w = np.asarray(qkv_w, np.float32)
    qkv_b = np.asarray(qkv_b, np.float32)
    out_w = np.asarray(out_w, np.float32)
    out_b = np.asarray(out_b, np.float32)
    norm_w = np.asarray(norm_w, np.float32)
    norm_b = np.asarray(norm_b, np.float32)

    wT = np.ascontiguousarray(qkv_w.T)          # [256, 768] in-ch major
    wqk = wT[:, 0:512].copy()
    wqk[:, 0:256] *= SCALE
    bqk = qkv_b[0:512].copy()
    bqk[0:256] *= SCALE
    wv = wT[:, 512:768]
    bv = qkv_b[512:768]
    woT = out_w.T                               # [256 in, 256 out]

    wall = np.zeros((128, 2, 1024), np.float32)
    for kc in range(2):
        wall[:, kc, 0:512] = wqk[128 * kc:128 * (kc + 1), :]
        wall[:, kc, 512:768] = wv[128 * kc:128 * (kc + 1), :]
        wall[:, kc, 768:1024] = woT[128 * kc:128 * (kc + 1), :]

    consts = np.zeros((128, CW), np.float32)
    consts[:, COL_NWB + 0] = norm_w[0:128]
    consts[:, COL_NWB + 1] = norm_w[128:256]
    consts[:, COL_NWB + 2] = norm_b[0:128]
    consts[:, COL_NWB + 3] = norm_b[128:256]
    p = np.arange(128)
    consts[p, COL_GIND + p // 8] = 1.0 / GROUP_SZ
    consts[p // 8, COL_GINDT + p] = 1.0  # rows 0:16
    for m in range(4):
        consts[:, COL_QKB + m] = bqk[128 * m:128 * (m + 1)]
    consts[:, COL_ESH] = ESHIFT
    consts[:, COL_EPS] = EPS

    # ybias[o, r] = sum_c Wo[o, c] * bv[(c//32)*32 + r] + out_b[o]
    bvpat = np.zeros((256, 32), np.float32)
    for c in range(256):
        bvpat[c, :] = bv[(c // 32) * 32 + np.arange(32)]
    ybias = out_w @ bvpat + out_b[:, None]      # [256, 32]

    constsb = np.zeros((128, CWB), np.float32)
    pp = np.arange(1024)
    constsb[pp % 32, CB_P32 + pp] = 1.0          # rows 0:32
    for ot in range(2):
        constsb[0:32, CB_YBT + 128 * ot:CB_YBT + 128 * (ot + 1)] = \
            ybias[128 * ot:128 * (ot + 1), :].T

    shared = {
        "wall": wall.astype(ml_dtypes.bfloat16),
        "consts": consts,
        "constsb": constsb.astype(ml_dtypes.bfloat16),
        "zeros8": np.zeros((128, 2, HW), ml_dtypes.float8_e4m3),
    }
    in_maps = []
    for b in range(N_CORES):
        m = dict(shared)
        m["x_in"] = np.ascontiguousarray(
            x[b].reshape(2, 128, HW).transpose(1, 0, 2))
        in_maps.append(m)
    return in_maps


_PROGRAM = None


def _get_program():
    global _PROGRAM
    if _PROGRAM is None:
        _PROGRAM = build_program()
    return _PROGRAM


def kernel(x, norm_w, norm_b, qkv_w, qkv_b, out_w, out_b, _trace=False):
    nc = _get_program()
    in_maps = host_prep(x, norm_w, norm_b, qkv_w, qkv_b, out_w, out_b)
    res = run_bass_kernel_spmd(nc, in_maps, list(range(N_CORES)), trace=_trace)
    out = np.stack([res.results[b]["y_out"].reshape(C, H, W)
                    for b in range(N_CORES)])
    if _trace:
        kernel.last_result = res
    return out.astype(np.float32)
